# revision 13
# baseline (speedup 1.0000x reference)
"""EquivariantEvolution kernel for 8 Trainium2 NeuronCores (Bass/Tile).

Math (per sample, reference):
    alpha = Linear2(silu(Linear1(z)))                     # [NG]
    A     = sum_g alpha_g G_g                             # [D, D]
    z_t   = (I + A + A^2/2 + A^3/6 + A^4/24) z            # order-4 Taylor
    h1    = W1 z_t + b1
    out   = W2 (sigmoid(|h1| + eps) * h1) + b2

Device strategy (pure data-parallel over batch, feature-major layout):
  * Host pre-transposes z to [D, B/8] bf16 per core; weights are reshaped
    on host so the device runs only matmuls / elementwise ops, all in bf16
    (PSUM accumulation stays fp32).
  * Horner:  v <- z + (1/k) A v.  A v is one K=128 contraction:
      y[(g,i), b] = alpha_g[b] * v[i, b]   (elementwise; alpha replicated
      across the 32 i-partitions by construction)
      (A v)[j, b] = sum_{(g,i)} G[g,j,i] y[(g,i),b]   (two K=128 matmuls)
    lhsT is tiled 4x along M so the output lands pre-replicated for the
    next step's elementwise multiply; the z-add is a third K=32 matmul.
  * Engine balance per tile: PE 16 matmul slots; DVE gets the two
    PSUM-sourced taylor multiplies plus one 2x-rate SBUF pair; ACT runs
    silu/alpha-bias/h1/one pv->bf16 staging copy; GpSimd runs the z-based
    multiply and the h1^2 square; Sync carries input DMA, GpSimd queue the
    output DMA.
  * HAM discipline: zero-weight matmul bursts open the PE clock gate
    (K=8/8) at startup and across the sqrt/tanh ACT-table-switch bubble;
    everything else is a dense bf16 matmul stream, so the gate stays open.
  * Gate: sigmoid(norm + eps) = 0.5 tanh(norm/2 + eps/2) + 0.5 is produced
    broadcast to [H, BT] by a single K=17 matmul (16 tanh rows + a
    constant-ones row carrying the +0.5).  Phase B is software-pipelined:
    gate matmul, DVE multiply, one output matmul per tile, outputs of 2
    tiles packed per PSUM bank before a single bias+store pass.
"""

import os
import sys

import numpy as np

for _p in ("/opt/trn_rl_repo", "/root/.axon_site/_ro/trn_rl_repo"):
    if os.path.isdir(_p) and _p not in sys.path:
        sys.path.insert(0, _p)

import concourse.bass as bass
import concourse.mybir as mybir
import concourse.tile as tile
from concourse.bass_utils import run_bass_kernel_spmd

B, D, H, NG = 65536, 32, 128, 8
NCORES = 8
BC = B // NCORES          # samples per core
BT = 512                  # samples (free-dim columns) per tile
EPS = 1e-6
F32 = mybir.dt.float32
BF16 = mybir.dt.bfloat16
AF = mybir.ActivationFunctionType

# bf16 weight blob layouts: name -> (col offset, n cols); all [128, .] in
# CAT128, all [32, .] in CAT32.  Order matters only for the host packer.
_CAT128 = [
    ("LT_At", H), ("LT_Ab", H),
    ("LT_t4", H), ("LT_b4", H), ("LT_t3", H), ("LT_b3", H),
    ("LT_t2", H), ("LT_b2k", H), ("LT_t1", H), ("LT_b1k", H),
    ("LT_W2", D),
]
_CAT32 = [("LT_h", H), ("LT_z", H), ("LT_W1z", H)]
_CATB = [("Bse1", 1), ("Bse2t", 1), ("Bse2b", 1), ("B1", 1)]  # [128,1] f32


def _cat_cols(cat):
    off, out = 0, {}
    for name, w in cat:
        out[name] = (off, w)
        off += w
    return out, off


def _split_multi_waits(nc, max_waits=1):
    """This toolchain's walrus rejects >1 sync-wait on an instruction
    ("Too many sync wait commands"); hoist extra waits onto preceding
    same-engine NOPs (in-order engines make this semantics-preserving)."""
    n_new = 0
    for f in nc.m.functions:
        for bb in f.blocks:
            out = []
            for ins in bb.instructions:
                si = getattr(ins, "sync_info", None)
                if si is not None and si.on_wait and len(si.on_wait) > max_waits:
                    waits = list(si.on_wait)
                    chunks = [waits[i:i + max_waits] for i in range(0, len(waits), max_waits)]
                    for ci, ch in enumerate(chunks[:-1]):
                        nop = mybir.InstNoOp(
                            name=f"{ins.name}-wsplit{ci}",
                            engine=ins.engine,
                            sync_info=mybir.SyncInfo(on_wait=ch, on_update=[]),
                            bass_nofuse=True,
                        )
                        out.append(nop)
                        n_new += 1
                    ins.sync_info = mybir.SyncInfo(on_wait=chunks[-1], on_update=si.on_update)
                out.append(ins)
            bb.instructions[:] = out
    return n_new


def _build_program(bc: int, sim_safe: bool = False, split_waits: bool = True):
    """Trace the per-core Bass program for bc samples.

    sim_safe decomposes Silu into Sigmoid*x (CoreSim has no Silu handler);
    the hardware path uses the native Silu LUT.
    """
    nt = bc // BT
    ng2 = nt // 2             # output 2-tile groups
    nc = bass.Bass()

    c128_cols, c128_w = _cat_cols(_CAT128)
    c32_cols, c32_w = _cat_cols(_CAT32)
    cb_cols, cb_w = _cat_cols(_CATB)

    zT = nc.declare_dram_parameter("zT", [D, bc], BF16, isOutput=False)
    cat128 = nc.declare_dram_parameter("CAT128", [H, c128_w], BF16, isOutput=False)
    cat32 = nc.declare_dram_parameter("CAT32", [D, c32_w], BF16, isOutput=False)
    catb = nc.declare_dram_parameter("CATB", [H, cb_w], F32, isOutput=False)
    onsq = nc.declare_dram_parameter("ONSQ", [H, nt * nt], BF16, isOutput=False)
    esig = nc.declare_dram_parameter("E_sig", [nt + 1, nt * H], BF16, isOutput=False)
    b22 = nc.declare_dram_parameter("B2_2", [2 * D, 1], F32, isOutput=False)
    outT = nc.declare_dram_parameter("outT", [2 * D, ng2 * BT], F32, isOutput=True)

    with tile.TileContext(nc) as tc:
        with (
            tc.tile_pool(name="consts", bufs=1) as consts,
            tc.tile_pool(name="zv4", bufs=3) as zv4_pool,
            tc.tile_pool(name="hs", bufs=3) as hs_pool,
            tc.tile_pool(name="acat", bufs=6) as acat_pool,
            tc.tile_pool(name="ycat", bufs=4) as ycat_pool,
            tc.tile_pool(name="pvb", bufs=3) as pvb_pool,
            tc.tile_pool(name="sq", bufs=4) as sq_pool,
            tc.tile_pool(name="h1s", bufs=nt) as h1s_pool,
            tc.tile_pool(name="gate", bufs=1) as gate_pool,
            tc.tile_pool(name="a1g", bufs=3) as a1g_pool,
            tc.tile_pool(name="outs", bufs=2) as outs_pool,
            tc.tile_pool(name="ps", bufs=3, space=bass.MemorySpace.PSUM) as ps_pool,
            tc.tile_pool(name="pv", bufs=3, space=bass.MemorySpace.PSUM) as pv_pool,
            tc.tile_pool(name="psn", bufs=1, space=bass.MemorySpace.PSUM) as psn_pool,
            tc.tile_pool(name="warm", bufs=1, space=bass.MemorySpace.PSUM) as warm_pool,
        ):
            # ---- HAM trigger: zero-weight matmuls need no DMA'd data, so
            # the PE clock gate opens while constants stream in ----
            wscr = consts.tile([H, BT], BF16, name="wscr")
            nc.vector.memset(wscr[:], 0.0)
            wps = warm_pool.tile([H, BT], F32, name="wps", tag="warm")

            def warm(n, cols=BT):
                for _ in range(n):
                    nc.tensor.matmul(wps[:, 0:cols], wscr[:, 0:H], wscr[:, 0:cols],
                                     start=True, stop=True)

            warm(14)

            # ---- load constants into SBUF (few large DMAs) ----
            c128_t = consts.tile([H, c128_w], BF16, name="c_cat128")
            nc.sync.dma_start(c128_t[:], cat128[:])
            c32_t = consts.tile([D, c32_w], BF16, name="c_cat32")
            nc.sync.dma_start(c32_t[:], cat32[:])
            cb_t = consts.tile([H, cb_w], F32, name="c_catb")
            nc.sync.dma_start(cb_t[:], catb[:])
            onsq_t = consts.tile([H, nt * nt], BF16, name="c_onsq")
            nc.sync.dma_start(onsq_t[:], onsq[:])
            esig_t = consts.tile([nt + 1, nt * H], BF16, name="c_esig")
            nc.sync.dma_start(esig_t[:], esig[:])
            b22_t = consts.tile([2 * D, 1], F32, name="c_b22")
            nc.sync.dma_start(b22_t[:], b22[:])

            ct = {}
            for name, (off, w) in c128_cols.items():
                ct[name] = c128_t[:, off:off + w]
            for name, (off, w) in c32_cols.items():
                ct[name] = c32_t[:, off:off + w]
            for name, (off, w) in cb_cols.items():
                ct[name] = cb_t[:, off:off + w]

            zero_b = consts.tile([nt, 1], F32, name="zero_b")
            nc.vector.memset(zero_b[:], 0.0)
            tanh_b = consts.tile([nt, 1], F32, name="tanh_b")
            nc.vector.memset(tanh_b[:], 0.5 * EPS)
            # tanh rows 0..nt-1 + constant-ones row nt (the +0.5 path);
            # memset the whole tile (single-partition writes at base 16 are
            # rejected by the BIR verifier), tanh later overwrites rows 0..15
            t17 = consts.tile([nt + 1, BT], BF16, name="t17")
            nc.vector.memset(t17[:], 1.0)

            taylor = [
                (ct["LT_t4"], ct["LT_b4"]),
                (ct["LT_t3"], ct["LT_b3"]),
                (ct["LT_t2"], ct["LT_b2k"]),
            ]

            h1s_tiles = []
            nsq_ps = psn_pool.tile([nt, BT], F32, name="nsq_ps", tag="nsq")

            # ================= phase A =================
            # Software pipeline at pair granularity: emit the extractor
            # ("front") of pair p+1 before the taylor chain ("back") of
            # pair p, so the in-order ACT/GpSimd queues overlap pairs
            # instead of serializing on each pair's ~10us dependency chain.
            # Each pair's norm matmuls are deferred one back-stage so the PE
            # never stalls on the h1 -> square chain.
            fronts = {}
            pend_nsq = []

            def emit_front(p, fill):
                pair = [2 * p, 2 * p + 1]
                zv4 = zv4_pool.tile([H, 2 * BT], BF16, name="zv4")
                for r in range(4):
                    nc.sync.dma_start(
                        zv4[32 * r:32 * (r + 1), :],
                        zT[:, bass.ts(p, 2 * BT)],
                    )
                zsl = {t: zv4[:, bass.ts(t - 2 * p, BT)] for t in pair}

                hss, acats, ycats = {}, {}, {}
                for t in pair:
                    hp = ps_pool.tile([H, BT], F32, name="hp", tag="ps")
                    nc.tensor.matmul(hp[:], ct["LT_h"][:], zsl[t][0:D, :], start=True, stop=True)
                    hs = hs_pool.tile([H, BT], BF16, name="hs")
                    if sim_safe:
                        sg = hs_pool.tile([H, BT], F32, name="sg")
                        nc.scalar.activation(sg[:], hp[:], AF.Sigmoid, bias=ct["Bse1"][:])
                        hx = hs_pool.tile([H, BT], F32, name="hx")
                        nc.scalar.activation(hx[:], hp[:], AF.Identity, bias=ct["Bse1"][:])
                        nc.vector.tensor_tensor(hs[:], sg[:], hx[:], mybir.AluOpType.mult)
                    else:
                        nc.scalar.activation(hs[:], hp[:], AF.Silu, bias=ct["Bse1"][:])
                    hss[t] = hs
                fill(2)

                for t in pair:
                    apt = ps_pool.tile([H, BT], F32, name="apt", tag="ps")
                    nc.tensor.matmul(apt[:], ct["LT_At"][:], hss[t][:], start=True, stop=True)
                    apb = ps_pool.tile([H, BT], F32, name="apb", tag="ps")
                    nc.tensor.matmul(apb[:], ct["LT_Ab"][:], hss[t][:], start=True, stop=True)
                    acat = acat_pool.tile([H, 2, BT], BF16, name="acat")
                    nc.scalar.activation(acat[:, 0, :], apt[:], AF.Identity, bias=ct["Bse2t"][:])
                    nc.scalar.activation(acat[:, 1, :], apb[:], AF.Identity, bias=ct["Bse2b"][:])
                    acats[t] = acat
                fill(4)

                for t in pair:
                    # all-SBUF bf16 multiply: runs on GpSimd to keep DVE free
                    # for the PSUM-sourced taylor multiplies
                    ycat = ycat_pool.tile([H, 2, BT], BF16, name="ycat0")
                    nc.gpsimd.tensor_tensor(
                        ycat[:], acats[t][:],
                        zsl[t][:, None, :].broadcast_to([H, 2, BT]),
                        mybir.AluOpType.mult,
                    )
                    ycats[t] = ycat
                fronts[p] = (pair, zsl, acats, ycats)

            def emit_nsq_pending():
                for t, sq in pend_nsq:
                    nc.tensor.matmul(
                        nsq_ps[:], onsq_t[:, bass.ts(t, nt)], sq[:],
                        start=(t == 0), stop=(t == nt - 1), skip_group_check=True,
                    )
                pend_nsq.clear()

            def emit_back(p):
                pair, zsl, acats, ycats = fronts.pop(p)
                emit_nsq_pending()

                for step_i, (lt_top, lt_bot) in enumerate(taylor):
                    pvs = {}
                    for t in pair:
                        pv = pv_pool.tile([H, BT], F32, name="pv", tag="pv")
                        nc.tensor.matmul(pv[:], lt_top[:], ycats[t][:, 0, :], start=True, stop=False)
                        nc.tensor.matmul(pv[:], lt_bot[:], ycats[t][:, 1, :], start=False, stop=False)
                        nc.tensor.matmul(pv[:], ct["LT_z"][:], zsl[t][0:D, :], start=False, stop=True)
                        pvs[t] = pv
                    if step_i == 0:
                        # stage pv through bf16 SBUF (ACT copy) so this
                        # step's multiplies run at DVE 2x as two 2D ops
                        for t in pair:
                            pvb = pvb_pool.tile([H, BT], BF16, name="pvb")
                            nc.scalar.activation(pvb[:], pvs[t][:], AF.Identity)
                            ycat = ycat_pool.tile([H, 2, BT], BF16, name="ycat")
                            nc.vector.tensor_tensor(
                                ycat[:, 0, :], acats[t][:, 0, :], pvb[:], mybir.AluOpType.mult)
                            nc.vector.tensor_tensor(
                                ycat[:, 1, :], acats[t][:, 1, :], pvb[:], mybir.AluOpType.mult)
                            ycats[t] = ycat
                    else:
                        for t in pair:
                            ycat = ycat_pool.tile([H, 2, BT], BF16, name="ycat")
                            nc.vector.tensor_tensor(
                                ycat[:], acats[t][:],
                                pvs[t][:, None, :].broadcast_to([H, 2, BT]),
                                mybir.AluOpType.mult,
                            )
                            ycats[t] = ycat

                h1ps = {}
                for t in pair:
                    h1p = pv_pool.tile([H, BT], F32, name="h1p", tag="pv")
                    nc.tensor.matmul(h1p[:], ct["LT_t1"][:], ycats[t][:, 0, :], start=True, stop=False)
                    nc.tensor.matmul(h1p[:], ct["LT_b1k"][:], ycats[t][:, 1, :], start=False, stop=False)
                    nc.tensor.matmul(h1p[:], ct["LT_W1z"][:], zsl[t][0:D, :], start=False, stop=True)
                    h1ps[t] = h1p

                for t in pair:
                    h1s = h1s_pool.tile([H, BT], BF16, name="h1s")
                    nc.scalar.activation(h1s[:], h1ps[t][:], AF.Identity, bias=ct["B1"][:])
                    h1s_tiles.append(h1s)
                    # square on GpSimd from the bf16 h1 (both operands SBUF)
                    sq = sq_pool.tile([H, BT], BF16, name="sq")
                    nc.gpsimd.tensor_tensor(sq[:], h1s[:], h1s[:], mybir.AluOpType.mult)
                    pend_nsq.append((t, sq))

            npairs = nt // 2
            for p in range(npairs):
                fill = (lambda n: warm(n)) if p == 0 else (lambda n: None)
                emit_front(p, fill)
                if p >= 1:
                    emit_back(p - 1)
            emit_back(npairs - 1)
            emit_nsq_pending()

            # ============== gate (batched sqrt + tanh) ==============
            # the two ACT table switches (~2.7us each) are the one PE bubble;
            # bridge it so the clock gate stays at K=8/8 for phase B
            warm(34)
            rt_all = gate_pool.tile([nt, BT], F32, name="rt_all")
            nc.scalar.activation(rt_all[:], nsq_ps[:], AF.Sqrt, bias=zero_b[:])
            # sigmoid(norm + eps) = 0.5 tanh(0.5 norm + eps/2) + 0.5
            nc.scalar.activation(t17[0:nt, :], rt_all[:], AF.Tanh, bias=tanh_b[:], scale=0.5)

            # ================= phase B (software-pipelined) =================
            trps = {}

            def emit_trp(t):
                trp = ps_pool.tile([H, BT], F32, name="trp", tag="ps")
                nc.tensor.matmul(
                    trp[:], esig_t[:, bass.ts(t, H)], t17[:],
                    start=True, stop=True,
                )
                trps[t] = trp

            for t in range(min(3, nt)):
                emit_trp(t)
            outg = None
            for t in range(nt):
                g, r = divmod(t, 2)
                a1g = a1g_pool.tile([H, BT], BF16, name="a1g")
                nc.vector.tensor_tensor(
                    a1g[:], h1s_tiles[t][:], trps.pop(t)[:], mybir.AluOpType.mult
                )
                if r == 0:
                    outg = pv_pool.tile([H, BT], F32, name="outg", tag="pv")
                nc.tensor.matmul(
                    outg[32 * r:32 * (r + 1), :], ct["LT_W2"][:], a1g[:],
                    start=True, stop=True, skip_group_check=True,
                )
                if t + 3 < nt:
                    emit_trp(t + 3)
                if r == 1:
                    outs = outs_pool.tile([2 * D, BT], F32, name="outs")
                    nc.scalar.activation(outs[:], outg[0:2 * D, :], AF.Identity, bias=b22_t[:])
                    nc.gpsimd.dma_start(outT[:, bass.ts(g, BT)], outs[:])

    if split_waits:
        _split_multi_waits(nc)
    return nc


def _host_params(G, W_se1, b_se1, W_se2, b_se2, W1, b1, W2, b2, nt):
    import ml_dtypes
    f = np.float32
    bf = ml_dtypes.bfloat16
    G = np.asarray(G, f)
    Gflat = np.transpose(G, (0, 2, 1)).reshape(NG * D, D)  # [(g,i), j] = G[g,j,i]
    W1G = Gflat @ np.asarray(W1, f).T                      # [(g,i), m]
    e_sig = np.zeros((nt + 1, nt * H), f)
    for t in range(nt):
        e_sig[t, t * H:(t + 1) * H] = 0.5
    e_sig[nt, :] = 0.5
    w = {
        "LT_h": np.asarray(W_se1, f).T,
        "LT_At": np.repeat(np.asarray(W_se2, f).T[:, 0:4], 32, axis=1),
        "LT_Ab": np.repeat(np.asarray(W_se2, f).T[:, 4:8], 32, axis=1),
        "Bse1": np.asarray(b_se1, f).reshape(H, 1),
        "Bse2t": np.repeat(np.asarray(b_se2, f)[0:4], 32).reshape(H, 1),
        "Bse2b": np.repeat(np.asarray(b_se2, f)[4:8], 32).reshape(H, 1),
        "LT_z": np.tile(np.eye(D, dtype=f), (1, 4)),
        "LT_W1z": np.asarray(W1, f).T,
        "B1": np.asarray(b1, f).reshape(H, 1),
        "LT_W2": np.asarray(W2, f).T,
        "LT_t1": np.ascontiguousarray(W1G[:H]),
        "LT_b1k": np.ascontiguousarray(W1G[H:]),
    }
    for k, tname, bname in ((4, "LT_t4", "LT_b4"), (3, "LT_t3", "LT_b3"), (2, "LT_t2", "LT_b2k")):
        scaled = np.tile(Gflat * f(1.0 / k), (1, 4))
        w[tname] = np.ascontiguousarray(scaled[:H])
        w[bname] = np.ascontiguousarray(scaled[H:])
    p = {
        "CAT128": np.concatenate([w[n] for n, _ in _CAT128], axis=1).astype(bf),
        "CAT32": np.concatenate([w[n] for n, _ in _CAT32], axis=1).astype(bf),
        "CATB": np.concatenate([w[n] for n, _ in _CATB], axis=1).astype(f),
        "ONSQ": np.tile(np.eye(nt, dtype=f).reshape(1, nt * nt), (H, 1)).astype(bf),
        "E_sig": e_sig.astype(bf),
        "B2_2": np.tile(np.asarray(b2, f), 2).reshape(2 * D, 1).astype(f),
    }
    return {k: np.ascontiguousarray(v) for k, v in p.items()}


def _run(z, G, W_se1, b_se1, W_se2, b_se2, W1, b1, W2, b2, trace=False, **trace_kw):
    import ml_dtypes
    z = np.asarray(z, np.float32)
    nt = BC // BT
    params = _host_params(G, W_se1, b_se1, W_se2, b_se2, W1, b1, W2, b2, nt)

    # shard: per-core feature-major bf16 slices
    zT = np.ascontiguousarray(
        z.reshape(NCORES, BC, D).transpose(0, 2, 1).astype(ml_dtypes.bfloat16)
    )

    nc = _build_program(BC)
    in_maps = [{"zT": zT[c], **params} for c in range(NCORES)]
    res = run_bass_kernel_spmd(nc, in_maps, list(range(NCORES)), trace=trace, **trace_kw)

    # outT[32r+d, g*BT+b] = out[(2g+r)*BT + b, d] per core
    outT = np.stack([res.results[c]["outT"] for c in range(NCORES)])
    out = (
        outT.reshape(NCORES, 2, D, nt // 2, BT)
        .transpose(0, 3, 1, 4, 2)
        .reshape(B, D)
    )
    return np.ascontiguousarray(out.astype(np.float32)), res


def kernel(z, G, W_se1, b_se1, W_se2, b_se2, W1, b1, W2, b2):
    out, _ = _run(z, G, W_se1, b_se1, W_se2, b_se2, W1, b1, W2, b2, trace=False)
    return out


if __name__ == "__main__":
    rng = np.random.default_rng(0)
    inputs = {
        "z": rng.standard_normal((B, D), dtype=np.float32),
        "G": (rng.standard_normal((NG, D, D)) * 0.1).astype(np.float32),
        "W_se1": (rng.standard_normal((H, D)) / np.sqrt(D)).astype(np.float32),
        "b_se1": np.zeros(H, np.float32),
        "W_se2": (rng.standard_normal((NG, H)) / np.sqrt(H)).astype(np.float32),
        "b_se2": np.zeros(NG, np.float32),
        "W1": (rng.standard_normal((H, D)) * 0.01).astype(np.float32),
        "b1": np.zeros(H, np.float32),
        "W2": (rng.standard_normal((D, H)) * 0.01).astype(np.float32),
        "b2": np.zeros(D, np.float32),
    }
    out = kernel(**inputs)
    print("kernel output", out.shape, out.dtype, float(np.abs(out).max()))


# revision 18
# speedup vs baseline: 1.3784x; 1.3784x over previous
"""EquivariantEvolution kernel for 8 Trainium2 NeuronCores (Bass/Tile).

Math (per sample, reference):
    alpha = Linear2(silu(Linear1(z)))                     # [NG]
    A     = sum_g alpha_g G_g                             # [D, D]
    z_t   = (I + A + A^2/2 + A^3/6 + A^4/24) z            # order-4 Taylor
    h1    = W1 z_t + b1
    out   = W2 (sigmoid(|h1| + eps) * h1) + b2

Device strategy (pure data-parallel over batch, feature-major layout):
  * Host pre-transposes z to [D, B/8] bf16 per core; weights are reshaped
    on host so the device runs only matmuls / elementwise ops, all in bf16
    (PSUM accumulation stays fp32).
  * Horner:  v <- z + (1/k) A v.  A v is one K=128 contraction:
      y[(g,i), b] = alpha_g[b] * v[i, b]   (elementwise; alpha replicated
      across the 32 i-partitions by construction)
      (A v)[j, b] = sum_{(g,i)} G[g,j,i] y[(g,i),b]   (two K=128 matmuls)
    lhsT is tiled 4x along M so the output lands pre-replicated for the
    next step's elementwise multiply; the z-add is a third K=32 matmul.
  * Engine balance per tile: PE 16 matmul slots; DVE gets the two
    PSUM-sourced taylor multiplies plus one 2x-rate SBUF pair; ACT runs
    silu/alpha-bias/h1/one pv->bf16 staging copy; GpSimd runs the z-based
    multiply and the h1^2 square; Sync carries input DMA, GpSimd queue the
    output DMA.
  * HAM discipline: zero-weight matmul bursts open the PE clock gate
    (K=8/8) at startup and across the sqrt/tanh ACT-table-switch bubble;
    everything else is a dense bf16 matmul stream, so the gate stays open.
  * Gate: sigmoid(norm + eps) = 0.5 tanh(norm/2 + eps/2) + 0.5 is produced
    broadcast to [H, BT] by a single K=17 matmul (16 tanh rows + a
    constant-ones row carrying the +0.5).  Phase B is software-pipelined:
    gate matmul, DVE multiply, one output matmul per tile, outputs of 2
    tiles packed per PSUM bank before a single bias+store pass.
"""

import os
import sys

import numpy as np

for _p in ("/opt/trn_rl_repo", "/root/.axon_site/_ro/trn_rl_repo"):
    if os.path.isdir(_p) and _p not in sys.path:
        sys.path.insert(0, _p)

import concourse.bass as bass
import concourse.mybir as mybir
import concourse.tile as tile
from concourse.bass_utils import run_bass_kernel_spmd

B, D, H, NG = 65536, 32, 128, 8
NCORES = 8
BC = B // NCORES          # samples per core
BT = 512                  # samples (free-dim columns) per tile
EPS = 1e-6
F32 = mybir.dt.float32
BF16 = mybir.dt.bfloat16
AF = mybir.ActivationFunctionType

# bf16 weight blob layouts: name -> (col offset, n cols); all [128, .] in
# CAT128, all [32, .] in CAT32.  Order matters only for the host packer.
_CAT128 = [
    ("LT_At", H), ("LT_Ab", H),
    ("LT_t4", H), ("LT_b4", H), ("LT_t3", H), ("LT_b3", H),
    ("LT_t2", H), ("LT_b2k", H), ("LT_t1", H), ("LT_b1k", H),
    ("LT_W2", D),
]
_CAT32 = [("LT_h", H), ("LT_z", H), ("LT_W1z", H)]
_CATB = [("Bse1", 1), ("Bse2t", 1), ("Bse2b", 1), ("B1", 1)]  # [128,1] f32


def _cat_cols(cat):
    off, out = 0, {}
    for name, w in cat:
        out[name] = (off, w)
        off += w
    return out, off


def _split_multi_waits(nc, max_waits=1):
    """This toolchain's walrus rejects >1 sync-wait on an instruction
    ("Too many sync wait commands"); hoist extra waits onto preceding
    same-engine NOPs (in-order engines make this semantics-preserving)."""
    n_new = 0
    for f in nc.m.functions:
        for bb in f.blocks:
            out = []
            for ins in bb.instructions:
                si = getattr(ins, "sync_info", None)
                if si is not None and si.on_wait and len(si.on_wait) > max_waits:
                    waits = list(si.on_wait)
                    chunks = [waits[i:i + max_waits] for i in range(0, len(waits), max_waits)]
                    for ci, ch in enumerate(chunks[:-1]):
                        nop = mybir.InstNoOp(
                            name=f"{ins.name}-wsplit{ci}",
                            engine=ins.engine,
                            sync_info=mybir.SyncInfo(on_wait=ch, on_update=[]),
                            bass_nofuse=True,
                        )
                        out.append(nop)
                        n_new += 1
                    ins.sync_info = mybir.SyncInfo(on_wait=chunks[-1], on_update=si.on_update)
                out.append(ins)
            bb.instructions[:] = out
    return n_new


def _build_program(bc: int, sim_safe: bool = False, split_waits: bool = True):
    """Trace the per-core Bass program for bc samples.

    sim_safe decomposes Silu into Sigmoid*x (CoreSim has no Silu handler);
    the hardware path uses the native Silu LUT.
    """
    nt = bc // BT
    ng2 = nt // 2             # output 2-tile groups
    nc = bass.Bass()

    c128_cols, c128_w = _cat_cols(_CAT128)
    c32_cols, c32_w = _cat_cols(_CAT32)
    cb_cols, cb_w = _cat_cols(_CATB)

    zT = nc.declare_dram_parameter("zT", [D, bc], BF16, isOutput=False)
    cat128 = nc.declare_dram_parameter("CAT128", [H, c128_w], BF16, isOutput=False)
    cat32 = nc.declare_dram_parameter("CAT32", [D, c32_w], BF16, isOutput=False)
    catb = nc.declare_dram_parameter("CATB", [H, cb_w], F32, isOutput=False)
    onsq = nc.declare_dram_parameter("ONSQ", [H, nt * nt], BF16, isOutput=False)
    esig = nc.declare_dram_parameter("E_sig", [nt + 1, nt * H], BF16, isOutput=False)
    b22 = nc.declare_dram_parameter("B2_2", [2 * D, 1], F32, isOutput=False)
    outT = nc.declare_dram_parameter("outT", [2 * D, ng2 * BT], F32, isOutput=True)

    with tile.TileContext(nc) as tc:
        with (
            tc.tile_pool(name="consts", bufs=1) as consts,
            tc.tile_pool(name="zv4", bufs=6) as zv4_pool,
            tc.tile_pool(name="hs", bufs=3) as hs_pool,
            tc.tile_pool(name="acat", bufs=6) as acat_pool,
            tc.tile_pool(name="ycat", bufs=8) as ycat_pool,
            tc.tile_pool(name="pvb", bufs=3) as pvb_pool,
            tc.tile_pool(name="sq", bufs=3) as sq_pool,
            tc.tile_pool(name="h1s", bufs=nt) as h1s_pool,
            tc.tile_pool(name="gate", bufs=1) as gate_pool,
            tc.tile_pool(name="a1g", bufs=3) as a1g_pool,
            tc.tile_pool(name="outs", bufs=2) as outs_pool,
            tc.tile_pool(name="ps", bufs=3, space=bass.MemorySpace.PSUM) as ps_pool,
            tc.tile_pool(name="pv", bufs=4, space=bass.MemorySpace.PSUM) as pv_pool,
            tc.tile_pool(name="psn", bufs=1, space=bass.MemorySpace.PSUM) as psn_pool,
        ):
            # ---- HAM trigger: zero-weight matmuls need no DMA'd data, so
            # the PE clock gate opens while constants stream in.  Warm
            # bursts borrow a pv-pool bank (freed back to rotation). ----
            wscr = consts.tile([H, BT], BF16, name="wscr")
            nc.vector.memset(wscr[:], 0.0)

            def warm(n, cols=BT):
                wps = pv_pool.tile([H, BT], F32, name="wps", tag="pv")
                for _ in range(n):
                    nc.tensor.matmul(wps[:, 0:cols], wscr[:, 0:H], wscr[:, 0:cols],
                                     start=True, stop=True)

            warm(14)

            # ---- load constants into SBUF (few large DMAs) ----
            c128_t = consts.tile([H, c128_w], BF16, name="c_cat128")
            nc.sync.dma_start(c128_t[:], cat128[:])
            c32_t = consts.tile([D, c32_w], BF16, name="c_cat32")
            nc.sync.dma_start(c32_t[:], cat32[:])
            cb_t = consts.tile([H, cb_w], F32, name="c_catb")
            nc.sync.dma_start(cb_t[:], catb[:])
            onsq_t = consts.tile([H, nt * nt], BF16, name="c_onsq")
            nc.sync.dma_start(onsq_t[:], onsq[:])
            esig_t = consts.tile([nt + 1, nt * H], BF16, name="c_esig")
            nc.sync.dma_start(esig_t[:], esig[:])
            b22_t = consts.tile([2 * D, 1], F32, name="c_b22")
            nc.sync.dma_start(b22_t[:], b22[:])

            ct = {}
            for name, (off, w) in c128_cols.items():
                ct[name] = c128_t[:, off:off + w]
            for name, (off, w) in c32_cols.items():
                ct[name] = c32_t[:, off:off + w]
            for name, (off, w) in cb_cols.items():
                ct[name] = cb_t[:, off:off + w]

            zero_b = consts.tile([nt, 1], F32, name="zero_b")
            nc.vector.memset(zero_b[:], 0.0)
            tanh_b = consts.tile([nt, 1], F32, name="tanh_b")
            nc.vector.memset(tanh_b[:], 0.5 * EPS)
            # tanh rows 0..nt-1 + constant-ones row nt (the +0.5 path);
            # memset the whole tile (single-partition writes at base 16 are
            # rejected by the BIR verifier), tanh later overwrites rows 0..15
            t17 = consts.tile([nt + 1, BT], BF16, name="t17")
            nc.vector.memset(t17[:], 1.0)

            taylor = [
                (ct["LT_t4"], ct["LT_b4"]),
                (ct["LT_t3"], ct["LT_b3"]),
                (ct["LT_t2"], ct["LT_b2k"]),
            ]

            h1s_tiles = [None] * nt
            nsq_ps = psn_pool.tile([nt, BT], F32, name="nsq_ps", tag="nsq")

            # ================= phase A: wavefront schedule =================
            # The per-tile chain is split into 11 macro-stages; tile t runs
            # stage m at wave t+m, so at steady state every wave carries one
            # tile in each stage.  Ops are emitted per wave in per-engine
            # readiness order, which makes each in-order engine queue process
            # ops exactly as their inputs become available: PE ~16 matmul
            # slots, ACT 5 ops, DVE 4 ops, GpSimd 2 ops per wave -- all four
            # engines near their ~3.4us/tile budget with no cross-pair
            # serialization.
            st = {t: {} for t in range(nt)}  # per-tile in-flight tensors

            def mm_T(t, lt_top, lt_bot, key_in, with_z):
                s = st[t]
                pv = pv_pool.tile([H, BT], F32, name="pv", tag="pv")
                ycat = s[key_in]
                nc.tensor.matmul(pv[:], lt_top[:], ycat[:, 0, :], start=True, stop=False)
                nc.tensor.matmul(pv[:], lt_bot[:], ycat[:, 1, :], start=False,
                                 stop=not with_z)
                if with_z:
                    nc.tensor.matmul(pv[:], ct["LT_z"][:], s["z"][0:D, :],
                                     start=False, stop=True)
                return pv

            def w_T2mm(t):
                st[t]["pv2"] = mm_T(t, ct["LT_t2"], ct["LT_b2k"], "y3", True)

            def w_ybc(t, key_pv, key_out):
                s = st[t]
                ycat = ycat_pool.tile([H, 2, BT], BF16, name="ycat")
                nc.vector.tensor_tensor(
                    ycat[:], s["acat"][:],
                    s.pop(key_pv)[:, None, :].broadcast_to([H, 2, BT]),
                    mybir.AluOpType.mult,
                )
                s[key_out] = ycat

            def w_T4mm(t):
                st[t]["pv4"] = mm_T(t, ct["LT_t4"], ct["LT_b4"], "y0", True)

            def w_acat(t):
                s = st[t]
                acat = acat_pool.tile([H, 2, BT], BF16, name="acat")
                nc.scalar.activation(acat[:, 0, :], s.pop("apt")[:], AF.Identity,
                                     bias=ct["Bse2t"][:])
                nc.scalar.activation(acat[:, 1, :], s.pop("apb")[:], AF.Identity,
                                     bias=ct["Bse2b"][:])
                s["acat"] = acat

            def w_pvb(t):
                s = st[t]
                pvb = pvb_pool.tile([H, BT], BF16, name="pvb")
                nc.scalar.activation(pvb[:], s.pop("pv4")[:], AF.Identity)
                s["pvb"] = pvb

            def w_y2d(t):
                # bf16 SBUF multiply pair: DVE 2x mode on plain 2D APs
                s = st[t]
                pvb = s.pop("pvb")
                ycat = ycat_pool.tile([H, 2, BT], BF16, name="ycat")
                nc.vector.tensor_tensor(
                    ycat[:, 0, :], s["acat"][:, 0, :], pvb[:], mybir.AluOpType.mult)
                nc.vector.tensor_tensor(
                    ycat[:, 1, :], s["acat"][:, 1, :], pvb[:], mybir.AluOpType.mult)
                s["y4"] = ycat

            def w_T3mm(t):
                st[t]["pv3"] = mm_T(t, ct["LT_t3"], ct["LT_b3"], "y4", True)

            def w_H1mm(t):
                s = st[t]
                h1p = pv_pool.tile([H, BT], F32, name="h1p", tag="pv")
                ycat = s.pop("y2")
                nc.tensor.matmul(h1p[:], ct["LT_t1"][:], ycat[:, 0, :], start=True, stop=False)
                nc.tensor.matmul(h1p[:], ct["LT_b1k"][:], ycat[:, 1, :], start=False, stop=False)
                nc.tensor.matmul(h1p[:], ct["LT_W1z"][:], s["z"][0:D, :],
                                 start=False, stop=True)
                s["h1p"] = h1p
                s.pop("acat")

            def w_h1s(t):
                s = st[t]
                h1s = h1s_pool.tile([H, BT], BF16, name="h1s")
                nc.scalar.activation(h1s[:], s.pop("h1p")[:], AF.Identity, bias=ct["B1"][:])
                h1s_tiles[t] = h1s

            def w_sq(t):
                # square on GpSimd from the bf16 h1 (both operands SBUF)
                sq = sq_pool.tile([H, BT], BF16, name="sq")
                nc.gpsimd.tensor_tensor(sq[:], h1s_tiles[t][:], h1s_tiles[t][:],
                                        mybir.AluOpType.mult)
                st[t]["sq"] = sq

            def w_nsq(t):
                nc.tensor.matmul(
                    nsq_ps[:], onsq_t[:, bass.ts(t, nt)], st[t].pop("sq")[:],
                    start=(t == 0), stop=(t == nt - 1), skip_group_check=True,
                )

            def w_apmm(t):
                s = st[t]
                apt = ps_pool.tile([H, BT], F32, name="apt", tag="ps")
                nc.tensor.matmul(apt[:], ct["LT_At"][:], s["hs"][:], start=True, stop=True)
                apb = ps_pool.tile([H, BT], F32, name="apb", tag="ps")
                nc.tensor.matmul(apb[:], ct["LT_Ab"][:], s.pop("hs")[:], start=True, stop=True)
                s["apt"], s["apb"] = apt, apb

            def w_hpmm(t):
                s = st[t]
                hp = ps_pool.tile([H, BT], F32, name="hp", tag="ps")
                nc.tensor.matmul(hp[:], ct["LT_h"][:], s["z"][0:D, :], start=True, stop=True)
                s["hp"] = hp

            def w_silu(t):
                s = st[t]
                hp = s.pop("hp")
                hs = hs_pool.tile([H, BT], BF16, name="hs")
                if sim_safe:
                    sg = hs_pool.tile([H, BT], F32, name="sg")
                    nc.scalar.activation(sg[:], hp[:], AF.Sigmoid, bias=ct["Bse1"][:])
                    hx = hs_pool.tile([H, BT], F32, name="hx")
                    nc.scalar.activation(hx[:], hp[:], AF.Identity, bias=ct["Bse1"][:])
                    nc.vector.tensor_tensor(hs[:], sg[:], hx[:], mybir.AluOpType.mult)
                else:
                    nc.scalar.activation(hs[:], hp[:], AF.Silu, bias=ct["Bse1"][:])
                s["hs"] = hs

            def w_ycat0(t):
                # all-SBUF bf16 multiply: runs on GpSimd to keep DVE free
                # for the PSUM-sourced taylor multiplies
                s = st[t]
                ycat = ycat_pool.tile([H, 2, BT], BF16, name="ycat0")
                nc.gpsimd.tensor_tensor(
                    ycat[:], s["acat"][:],
                    s["z"][:, None, :].broadcast_to([H, 2, BT]),
                    mybir.AluOpType.mult,
                )
                s["y0"] = ycat

            def w_dma(t):
                # pair-shared input load (4 replica DMAs per 2 tiles)
                zv4 = zv4_pool.tile([H, 2 * BT], BF16, name="zv4")
                for r in range(4):
                    nc.sync.dma_start(
                        zv4[32 * r:32 * (r + 1), :],
                        zT[:, bass.ts(t // 2, 2 * BT)],
                    )
                st[t]["z"] = zv4[:, 0:BT]
                st[t + 1]["z"] = zv4[:, BT:2 * BT]

            # wave w, tile w-m runs macro-stage m:
            #  m0 dma | m1 hp,silu | m2 ap | m3 acat | m4 ycat0
            #  m5 T4,pvb,y2d | m6 T3 | m7 ybc(pv3) | m8 T2,ybc(pv2)
            #  m9 H1,h1s | m10 sq | m11 nsq
            def alive(m):
                t = w - m
                return t if 0 <= t < nt else None

            for w in range(nt + 12):
                if 1 <= w <= 7:
                    warm(2)
                if (t := alive(0)) is not None and t % 2 == 0:
                    w_dma(t)
                if (t := alive(8)) is not None:
                    w_T2mm(t)
                if (t := alive(7)) is not None:
                    w_ybc(t, "pv3", "y3")
                if (t := alive(8)) is not None:
                    w_ybc(t, "pv2", "y2")
                if (t := alive(5)) is not None:
                    w_T4mm(t)
                if (t := alive(3)) is not None:
                    w_acat(t)
                if (t := alive(5)) is not None:
                    w_pvb(t)
                    w_y2d(t)
                if (t := alive(6)) is not None:
                    w_T3mm(t)
                if (t := alive(9)) is not None:
                    w_H1mm(t)
                    w_h1s(t)
                if (t := alive(4)) is not None:
                    w_ycat0(t)
                if (t := alive(10)) is not None:
                    w_sq(t)
                if (t := alive(11)) is not None:
                    w_nsq(t)
                if (t := alive(2)) is not None:
                    w_apmm(t)
                if (t := alive(1)) is not None:
                    w_hpmm(t)
                    w_silu(t)

            # ============== gate (batched sqrt + tanh) ==============
            # the two ACT table switches (~2.7us each) are the one PE bubble;
            # bridge it so the clock gate stays at K=8/8 for phase B
            warm(34)
            rt_all = gate_pool.tile([nt, BT], F32, name="rt_all")
            nc.scalar.activation(rt_all[:], nsq_ps[:], AF.Sqrt, bias=zero_b[:])
            # sigmoid(norm + eps) = 0.5 tanh(0.5 norm + eps/2) + 0.5
            nc.scalar.activation(t17[0:nt, :], rt_all[:], AF.Tanh, bias=tanh_b[:], scale=0.5)

            # ================= phase B (software-pipelined) =================
            trps = {}

            def emit_trp(t):
                trp = ps_pool.tile([H, BT], F32, name="trp", tag="ps")
                nc.tensor.matmul(
                    trp[:], esig_t[:, bass.ts(t, H)], t17[:],
                    start=True, stop=True,
                )
                trps[t] = trp

            for t in range(min(3, nt)):
                emit_trp(t)
            outg = None
            for t in range(nt):
                g, r = divmod(t, 2)
                a1g = a1g_pool.tile([H, BT], BF16, name="a1g")
                nc.vector.tensor_tensor(
                    a1g[:], h1s_tiles[t][:], trps.pop(t)[:], mybir.AluOpType.mult
                )
                if r == 0:
                    outg = pv_pool.tile([H, BT], F32, name="outg", tag="pv")
                nc.tensor.matmul(
                    outg[32 * r:32 * (r + 1), :], ct["LT_W2"][:], a1g[:],
                    start=True, stop=True, skip_group_check=True,
                )
                if t + 3 < nt:
                    emit_trp(t + 3)
                if r == 1:
                    outs = outs_pool.tile([2 * D, BT], F32, name="outs")
                    nc.scalar.activation(outs[:], outg[0:2 * D, :], AF.Identity, bias=b22_t[:])
                    nc.gpsimd.dma_start(outT[:, bass.ts(g, BT)], outs[:])

    if split_waits:
        _split_multi_waits(nc)
    return nc


def _host_params(G, W_se1, b_se1, W_se2, b_se2, W1, b1, W2, b2, nt):
    import ml_dtypes
    f = np.float32
    bf = ml_dtypes.bfloat16
    G = np.asarray(G, f)
    Gflat = np.transpose(G, (0, 2, 1)).reshape(NG * D, D)  # [(g,i), j] = G[g,j,i]
    W1G = Gflat @ np.asarray(W1, f).T                      # [(g,i), m]
    e_sig = np.zeros((nt + 1, nt * H), f)
    for t in range(nt):
        e_sig[t, t * H:(t + 1) * H] = 0.5
    e_sig[nt, :] = 0.5
    w = {
        "LT_h": np.asarray(W_se1, f).T,
        "LT_At": np.repeat(np.asarray(W_se2, f).T[:, 0:4], 32, axis=1),
        "LT_Ab": np.repeat(np.asarray(W_se2, f).T[:, 4:8], 32, axis=1),
        "Bse1": np.asarray(b_se1, f).reshape(H, 1),
        "Bse2t": np.repeat(np.asarray(b_se2, f)[0:4], 32).reshape(H, 1),
        "Bse2b": np.repeat(np.asarray(b_se2, f)[4:8], 32).reshape(H, 1),
        "LT_z": np.tile(np.eye(D, dtype=f), (1, 4)),
        "LT_W1z": np.asarray(W1, f).T,
        "B1": np.asarray(b1, f).reshape(H, 1),
        "LT_W2": np.asarray(W2, f).T,
        "LT_t1": np.ascontiguousarray(W1G[:H]),
        "LT_b1k": np.ascontiguousarray(W1G[H:]),
    }
    for k, tname, bname in ((4, "LT_t4", "LT_b4"), (3, "LT_t3", "LT_b3"), (2, "LT_t2", "LT_b2k")):
        scaled = np.tile(Gflat * f(1.0 / k), (1, 4))
        w[tname] = np.ascontiguousarray(scaled[:H])
        w[bname] = np.ascontiguousarray(scaled[H:])
    p = {
        "CAT128": np.concatenate([w[n] for n, _ in _CAT128], axis=1).astype(bf),
        "CAT32": np.concatenate([w[n] for n, _ in _CAT32], axis=1).astype(bf),
        "CATB": np.concatenate([w[n] for n, _ in _CATB], axis=1).astype(f),
        "ONSQ": np.tile(np.eye(nt, dtype=f).reshape(1, nt * nt), (H, 1)).astype(bf),
        "E_sig": e_sig.astype(bf),
        "B2_2": np.tile(np.asarray(b2, f), 2).reshape(2 * D, 1).astype(f),
    }
    return {k: np.ascontiguousarray(v) for k, v in p.items()}


def _run(z, G, W_se1, b_se1, W_se2, b_se2, W1, b1, W2, b2, trace=False, **trace_kw):
    import ml_dtypes
    z = np.asarray(z, np.float32)
    nt = BC // BT
    params = _host_params(G, W_se1, b_se1, W_se2, b_se2, W1, b1, W2, b2, nt)

    # shard: per-core feature-major bf16 slices
    zT = np.ascontiguousarray(
        z.reshape(NCORES, BC, D).transpose(0, 2, 1).astype(ml_dtypes.bfloat16)
    )

    nc = _build_program(BC)
    in_maps = [{"zT": zT[c], **params} for c in range(NCORES)]
    res = run_bass_kernel_spmd(nc, in_maps, list(range(NCORES)), trace=trace, **trace_kw)

    # outT[32r+d, g*BT+b] = out[(2g+r)*BT + b, d] per core
    outT = np.stack([res.results[c]["outT"] for c in range(NCORES)])
    out = (
        outT.reshape(NCORES, 2, D, nt // 2, BT)
        .transpose(0, 3, 1, 4, 2)
        .reshape(B, D)
    )
    return np.ascontiguousarray(out.astype(np.float32)), res


def kernel(z, G, W_se1, b_se1, W_se2, b_se2, W1, b1, W2, b2):
    out, _ = _run(z, G, W_se1, b_se1, W_se2, b_se2, W1, b1, W2, b2, trace=False)
    return out


if __name__ == "__main__":
    rng = np.random.default_rng(0)
    inputs = {
        "z": rng.standard_normal((B, D), dtype=np.float32),
        "G": (rng.standard_normal((NG, D, D)) * 0.1).astype(np.float32),
        "W_se1": (rng.standard_normal((H, D)) / np.sqrt(D)).astype(np.float32),
        "b_se1": np.zeros(H, np.float32),
        "W_se2": (rng.standard_normal((NG, H)) / np.sqrt(H)).astype(np.float32),
        "b_se2": np.zeros(NG, np.float32),
        "W1": (rng.standard_normal((H, D)) * 0.01).astype(np.float32),
        "b1": np.zeros(H, np.float32),
        "W2": (rng.standard_normal((D, H)) * 0.01).astype(np.float32),
        "b2": np.zeros(D, np.float32),
    }
    out = kernel(**inputs)
    print("kernel output", out.shape, out.dtype, float(np.abs(out).max()))


# revision 30
# speedup vs baseline: 1.4002x; 1.0158x over previous
"""EquivariantEvolution kernel for 8 Trainium2 NeuronCores (Bass/Tile).

Math (per sample, reference):
    alpha = Linear2(silu(Linear1(z)))                     # [NG]
    A     = sum_g alpha_g G_g                             # [D, D]
    z_t   = (I + A + A^2/2 + A^3/6 + A^4/24) z            # order-4 Taylor
    h1    = W1 z_t + b1
    out   = W2 (sigmoid(|h1| + eps) * h1) + b2

Device strategy (pure data-parallel over batch, feature-major layout):
  * Host pre-transposes z to [D, B/8] bf16 per core; weights are reshaped
    on host so the device runs only matmuls / elementwise ops, all in bf16
    (PSUM accumulation stays fp32).
  * Horner:  v <- z + (1/k) A v.  A v is one K=128 contraction:
      y[(g,i), b] = alpha_g[b] * v[i, b]   (elementwise; alpha replicated
      across the 32 i-partitions by construction)
      (A v)[j, b] = sum_{(g,i)} G[g,j,i] y[(g,i),b]   (two K=128 matmuls)
    lhsT is tiled 4x along M so the output lands pre-replicated for the
    next step's elementwise multiply; the z-add is a third K=32 matmul.
  * Engine balance per tile: PE 16 matmul slots; DVE gets the two
    PSUM-sourced taylor multiplies plus one 2x-rate SBUF pair; ACT runs
    silu/alpha-bias/h1/one pv->bf16 staging copy; GpSimd runs the z-based
    multiply and the h1^2 square; Sync carries input DMA, GpSimd queue the
    output DMA.
  * HAM discipline: zero-weight matmul bursts open the PE clock gate
    (K=8/8) at startup and across the sqrt/tanh ACT-table-switch bubble;
    everything else is a dense bf16 matmul stream, so the gate stays open.
  * Gate: sigmoid(norm + eps) = 0.5 tanh(norm/2 + eps/2) + 0.5 is produced
    broadcast to [H, BT] by a single K=17 matmul (16 tanh rows + a
    constant-ones row carrying the +0.5).  Phase B is software-pipelined:
    gate matmul, DVE multiply, one output matmul per tile, outputs of 2
    tiles packed per PSUM bank before a single bias+store pass.
"""

import os
import sys

import numpy as np

for _p in ("/opt/trn_rl_repo", "/root/.axon_site/_ro/trn_rl_repo"):
    if os.path.isdir(_p) and _p not in sys.path:
        sys.path.insert(0, _p)

import concourse.bass as bass
import concourse.mybir as mybir
import concourse.tile as tile
from concourse.bass_utils import run_bass_kernel_spmd

B, D, H, NG = 65536, 32, 128, 8
NCORES = 8
BC = B // NCORES          # samples per core
BT = 512                  # samples (free-dim columns) per tile
EPS = 1e-6
F32 = mybir.dt.float32
BF16 = mybir.dt.bfloat16
AF = mybir.ActivationFunctionType

# bf16 weight blob layouts: name -> (col offset, n cols); all [128, .] in
# CAT128, all [32, .] in CAT32.  Order matters only for the host packer.
_CAT128 = [
    ("LT_At", H), ("LT_Ab", H),
    ("LT_t4", H), ("LT_b4", H), ("LT_t3", H), ("LT_b3", H),
    ("LT_t2", H), ("LT_b2k", H), ("LT_t1", H), ("LT_b1k", H),
    ("LT_W2", D),
]
_CAT32 = [("LT_h", H), ("LT_z", H), ("LT_W1z", H)]
_CATB = [("Bse1", 1), ("Bse2t", 1), ("Bse2b", 1), ("B1", 1)]  # [128,1] f32


def _cat_cols(cat):
    off, out = 0, {}
    for name, w in cat:
        out[name] = (off, w)
        off += w
    return out, off


def _split_multi_waits(nc, max_waits=1):
    """This toolchain's walrus rejects >1 sync-wait on an instruction
    ("Too many sync wait commands"); hoist extra waits onto preceding
    same-engine NOPs (in-order engines make this semantics-preserving)."""
    n_new = 0
    for f in nc.m.functions:
        for bb in f.blocks:
            out = []
            for ins in bb.instructions:
                si = getattr(ins, "sync_info", None)
                if si is not None and si.on_wait and len(si.on_wait) > max_waits:
                    waits = list(si.on_wait)
                    chunks = [waits[i:i + max_waits] for i in range(0, len(waits), max_waits)]
                    for ci, ch in enumerate(chunks[:-1]):
                        nop = mybir.InstNoOp(
                            name=f"{ins.name}-wsplit{ci}",
                            engine=ins.engine,
                            sync_info=mybir.SyncInfo(on_wait=ch, on_update=[]),
                            bass_nofuse=True,
                        )
                        out.append(nop)
                        n_new += 1
                    ins.sync_info = mybir.SyncInfo(on_wait=chunks[-1], on_update=si.on_update)
                out.append(ins)
            bb.instructions[:] = out
    return n_new


def _build_program(bc: int, sim_safe: bool = False, split_waits: bool = True):
    """Trace the per-core Bass program for bc samples.

    sim_safe decomposes Silu into Sigmoid*x (CoreSim has no Silu handler);
    the hardware path uses the native Silu LUT.
    """
    nt = bc // BT
    ng2 = nt // 2             # output 2-tile groups
    nc = bass.Bass()

    c128_cols, c128_w = _cat_cols(_CAT128)
    c32_cols, c32_w = _cat_cols(_CAT32)
    cb_cols, cb_w = _cat_cols(_CATB)

    zT = nc.declare_dram_parameter("zT", [D, bc], BF16, isOutput=False)
    cat128 = nc.declare_dram_parameter("CAT128", [H, c128_w], BF16, isOutput=False)
    cat32 = nc.declare_dram_parameter("CAT32", [D, c32_w], BF16, isOutput=False)
    catb = nc.declare_dram_parameter("CATB", [H, cb_w], F32, isOutput=False)
    hnt = nt // 2
    onsq = nc.declare_dram_parameter("ONSQ", [H, nt * hnt], BF16, isOutput=False)
    esig = nc.declare_dram_parameter("E_sig", [hnt + 1, nt * H], BF16, isOutput=False)
    b22 = nc.declare_dram_parameter("B2_2", [2 * D, 1], F32, isOutput=False)
    outT = nc.declare_dram_parameter("outT", [2 * D, ng2 * BT], F32, isOutput=True)

    with tile.TileContext(nc) as tc:
        with (
            tc.tile_pool(name="consts", bufs=1) as consts,
            tc.tile_pool(name="zv4", bufs=7) as zv4_pool,
            tc.tile_pool(name="hs", bufs=4) as hs_pool,
            tc.tile_pool(name="acat", bufs=8) as acat_pool,
            tc.tile_pool(name="ycat", bufs=12) as ycat_pool,
            tc.tile_pool(name="pvb", bufs=4) as pvb_pool,
            tc.tile_pool(name="sq", bufs=4) as sq_pool,
            tc.tile_pool(name="h1s", bufs=nt) as h1s_pool,
            tc.tile_pool(name="gate", bufs=1) as gate_pool,
            tc.tile_pool(name="a1g", bufs=4) as a1g_pool,
            tc.tile_pool(name="outs", bufs=2) as outs_pool,
            tc.tile_pool(name="ps", bufs=3, space=bass.MemorySpace.PSUM) as ps_pool,
            tc.tile_pool(name="pv", bufs=4, space=bass.MemorySpace.PSUM) as pv_pool,
            tc.tile_pool(name="psn", bufs=1, space=bass.MemorySpace.PSUM) as psn_pool,
        ):
            # ---- HAM trigger: zero-weight matmuls need no DMA'd data, so
            # the PE clock gate opens while constants stream in.  Warm
            # bursts borrow a pv-pool bank (freed back to rotation). ----
            wscr = consts.tile([H, BT], BF16, name="wscr")
            nc.vector.memset(wscr[:], 0.0)

            def warm(n, cols=BT):
                wps = pv_pool.tile([H, BT], F32, name="wps", tag="pv")
                for _ in range(n):
                    nc.tensor.matmul(wps[:, 0:cols], wscr[:, 0:H], wscr[:, 0:cols],
                                     start=True, stop=True)

            warm(16)

            # ---- load constants into SBUF (few large DMAs) ----
            c128_t = consts.tile([H, c128_w], BF16, name="c_cat128")
            nc.sync.dma_start(c128_t[:], cat128[:])
            c32_t = consts.tile([D, c32_w], BF16, name="c_cat32")
            nc.sync.dma_start(c32_t[:], cat32[:])
            cb_t = consts.tile([H, cb_w], F32, name="c_catb")
            nc.sync.dma_start(cb_t[:], catb[:])
            onsq_t = consts.tile([H, nt * hnt], BF16, name="c_onsq")
            nc.sync.dma_start(onsq_t[:], onsq[:])
            esig_t = consts.tile([hnt + 1, nt * H], BF16, name="c_esig")
            nc.sync.dma_start(esig_t[:], esig[:])
            b22_t = consts.tile([2 * D, 1], F32, name="c_b22")
            nc.sync.dma_start(b22_t[:], b22[:])

            ct = {}
            for name, (off, w) in c128_cols.items():
                ct[name] = c128_t[:, off:off + w]
            for name, (off, w) in c32_cols.items():
                ct[name] = c32_t[:, off:off + w]
            for name, (off, w) in cb_cols.items():
                ct[name] = cb_t[:, off:off + w]

            zero_b = consts.tile([hnt, 1], F32, name="zero_b")
            nc.vector.memset(zero_b[:], 0.0)
            tanh_b = consts.tile([hnt, 1], F32, name="tanh_b")
            nc.vector.memset(tanh_b[:], 0.5 * EPS)
            # per-half-gate tanh rows 0..7 + constant-ones row 8 (the +0.5
            # path); memset whole tiles, tanh later overwrites rows 0..7
            t9a = consts.tile([hnt + 1, BT], BF16, name="t9a")
            nc.vector.memset(t9a[:], 1.0)
            t9b = consts.tile([hnt + 1, BT], BF16, name="t9b")
            nc.vector.memset(t9b[:], 1.0)

            taylor = [
                (ct["LT_t4"], ct["LT_b4"]),
                (ct["LT_t3"], ct["LT_b3"]),
                (ct["LT_t2"], ct["LT_b2k"]),
            ]

            h1s_tiles = [None] * nt
            # two independent accumulation groups in one bank: tiles 0..7 at
            # base partition 0, tiles 8..15 at base 32 (matmul output base
            # partitions must be 0/32/64), so the first sqrt can run while
            # the second half of phase A is still streaming
            nsq_ps = psn_pool.tile([40, BT], F32, name="nsq_ps", tag="nsq")

            # ================= phase A: wavefront schedule =================
            # The per-tile chain is split into 11 macro-stages; tile t runs
            # stage m at wave t+m, so at steady state every wave carries one
            # tile in each stage.  Ops are emitted per wave in per-engine
            # readiness order, which makes each in-order engine queue process
            # ops exactly as their inputs become available: PE ~16 matmul
            # slots, ACT 5 ops, DVE 4 ops, GpSimd 2 ops per wave -- all four
            # engines near their ~3.4us/tile budget with no cross-pair
            # serialization.
            st = {t: {} for t in range(nt)}  # per-tile in-flight tensors

            def mm_T(t, lt_top, lt_bot, key_in, with_z):
                s = st[t]
                pv = pv_pool.tile([H, BT], F32, name="pv", tag="pv")
                ycat = s[key_in]
                nc.tensor.matmul(pv[:], lt_top[:], ycat[:, 0, :], start=True, stop=False)
                nc.tensor.matmul(pv[:], lt_bot[:], ycat[:, 1, :], start=False,
                                 stop=not with_z)
                if with_z:
                    nc.tensor.matmul(pv[:], ct["LT_z"][:], s["z"][0:D, :],
                                     start=False, stop=True)
                return pv

            def w_T2mm(t):
                st[t]["pv2"] = mm_T(t, ct["LT_t2"], ct["LT_b2k"], "y3", True)

            def w_ybc(t, key_pv, key_out):
                s = st[t]
                ycat = ycat_pool.tile([H, 2, BT], BF16, name="ycat")
                nc.vector.tensor_tensor(
                    ycat[:], s["acat"][:],
                    s.pop(key_pv)[:, None, :].broadcast_to([H, 2, BT]),
                    mybir.AluOpType.mult,
                )
                s[key_out] = ycat

            def w_T4mm(t):
                st[t]["pv4"] = mm_T(t, ct["LT_t4"], ct["LT_b4"], "y0", True)

            def w_acat(t):
                s = st[t]
                acat = acat_pool.tile([H, 2, BT], BF16, name="acat")
                nc.scalar.activation(acat[:, 0, :], s.pop("apt")[:], AF.Identity,
                                     bias=ct["Bse2t"][:])
                nc.scalar.activation(acat[:, 1, :], s.pop("apb")[:], AF.Identity,
                                     bias=ct["Bse2b"][:])
                s["acat"] = acat

            def w_pvb(t):
                s = st[t]
                pvb = pvb_pool.tile([H, BT], BF16, name="pvb")
                nc.scalar.activation(pvb[:], s.pop("pv4")[:], AF.Identity)
                s["pvb"] = pvb

            def w_y2d(t):
                # all-bf16 SBUF multiply: one op, broadcast pvb over halves
                s = st[t]
                pvb = s.pop("pvb")
                ycat = ycat_pool.tile([H, 2, BT], BF16, name="ycat")
                nc.vector.tensor_tensor(
                    ycat[:], s["acat"][:],
                    pvb[:, None, :].broadcast_to([H, 2, BT]),
                    mybir.AluOpType.mult,
                )
                s["y4"] = ycat

            def w_T3mm(t):
                st[t]["pv3"] = mm_T(t, ct["LT_t3"], ct["LT_b3"], "y4", True)

            def w_H1mm(t):
                s = st[t]
                h1p = pv_pool.tile([H, BT], F32, name="h1p", tag="pv")
                ycat = s.pop("y2")
                nc.tensor.matmul(h1p[:], ct["LT_t1"][:], ycat[:, 0, :], start=True, stop=False)
                nc.tensor.matmul(h1p[:], ct["LT_b1k"][:], ycat[:, 1, :], start=False, stop=False)
                nc.tensor.matmul(h1p[:], ct["LT_W1z"][:], s["z"][0:D, :],
                                 start=False, stop=True)
                s["h1p"] = h1p
                s.pop("acat")

            def w_h1s(t):
                s = st[t]
                h1s = h1s_pool.tile([H, BT], BF16, name="h1s")
                nc.scalar.activation(h1s[:], s.pop("h1p")[:], AF.Identity, bias=ct["B1"][:])
                h1s_tiles[t] = h1s

            def w_sq(t):
                # square on GpSimd from the bf16 h1 (both operands SBUF)
                sq = sq_pool.tile([H, BT], BF16, name="sq")
                nc.gpsimd.tensor_tensor(sq[:], h1s_tiles[t][:], h1s_tiles[t][:],
                                        mybir.AluOpType.mult)
                st[t]["sq"] = sq

            def w_nsq(t):
                out = nsq_ps[0:hnt, :] if t < hnt else nsq_ps[32:32 + hnt, :]
                nc.tensor.matmul(
                    out, onsq_t[:, bass.ts(t, hnt)], st[t].pop("sq")[:],
                    start=(t % hnt == 0), stop=(t % hnt == hnt - 1),
                    skip_group_check=True,
                )

            def w_apmm(t):
                s = st[t]
                apt = ps_pool.tile([H, BT], F32, name="apt", tag="ps")
                nc.tensor.matmul(apt[:], ct["LT_At"][:], s["hs"][:], start=True, stop=True)
                apb = ps_pool.tile([H, BT], F32, name="apb", tag="ps")
                nc.tensor.matmul(apb[:], ct["LT_Ab"][:], s.pop("hs")[:], start=True, stop=True)
                s["apt"], s["apb"] = apt, apb

            def w_hpmm(t):
                s = st[t]
                hp = ps_pool.tile([H, BT], F32, name="hp", tag="ps")
                nc.tensor.matmul(hp[:], ct["LT_h"][:], s["z"][0:D, :], start=True, stop=True)
                s["hp"] = hp

            def w_silu(t):
                s = st[t]
                hp = s.pop("hp")
                hs = hs_pool.tile([H, BT], BF16, name="hs")
                if sim_safe:
                    sg = hs_pool.tile([H, BT], F32, name="sg")
                    nc.scalar.activation(sg[:], hp[:], AF.Sigmoid, bias=ct["Bse1"][:])
                    hx = hs_pool.tile([H, BT], F32, name="hx")
                    nc.scalar.activation(hx[:], hp[:], AF.Identity, bias=ct["Bse1"][:])
                    nc.vector.tensor_tensor(hs[:], sg[:], hx[:], mybir.AluOpType.mult)
                else:
                    nc.scalar.activation(hs[:], hp[:], AF.Silu, bias=ct["Bse1"][:])
                s["hs"] = hs

            def w_ycat0(t):
                # all-SBUF bf16 multiply: runs on GpSimd to keep DVE free
                # for the PSUM-sourced taylor multiplies
                s = st[t]
                ycat = ycat_pool.tile([H, 2, BT], BF16, name="ycat0")
                nc.gpsimd.tensor_tensor(
                    ycat[:], s["acat"][:],
                    s["z"][:, None, :].broadcast_to([H, 2, BT]),
                    mybir.AluOpType.mult,
                )
                s["y0"] = ycat

            def w_dma(t):
                # pair-shared input load (4 replica DMAs per 2 tiles)
                zv4 = zv4_pool.tile([H, 2 * BT], BF16, name="zv4")
                for r in range(4):
                    nc.sync.dma_start(
                        zv4[32 * r:32 * (r + 1), :],
                        zT[:, bass.ts(t // 2, 2 * BT)],
                    )
                st[t]["z"] = zv4[:, 0:BT]
                st[t + 1]["z"] = zv4[:, BT:2 * BT]

            # wave w, tile w-m runs macro-stage m:
            #  m0 dma | m1 hp,silu | m2 ap | m3 acat | m4 ycat0
            #  m5 T4,pvb,y2d | m6 T3 | m7 ybc(pv3) | m8 T2,ybc(pv2)
            #  m9 H1,h1s | m10 sq | m11 nsq
            def alive(m):
                t = w - m
                return t if 0 <= t < nt else None

            rt_a = gate_pool.tile([hnt, BT], F32, name="rt_a")
            rt_b = gate_pool.tile([hnt, BT], F32, name="rt_b")

            for w in range(nt + 12):
                if 1 <= w <= 9:
                    warm(3)
                if w == nt + 5:
                    # first-half norms closed (wave nt+3); the switch to the
                    # sqrt table and this sqrt hide under the phase A tail --
                    # the only remaining phase A ACT ops are Identity, which
                    # is filler in every table set
                    nc.scalar.activation(rt_a[:], nsq_ps[0:hnt, :], AF.Sqrt,
                                         bias=zero_b[:])
                if (t := alive(0)) is not None and t % 2 == 0:
                    w_dma(t)
                if (t := alive(8)) is not None:
                    w_T2mm(t)
                if (t := alive(7)) is not None:
                    w_ybc(t, "pv3", "y3")
                if (t := alive(8)) is not None:
                    w_ybc(t, "pv2", "y2")
                if (t := alive(5)) is not None:
                    w_T4mm(t)
                if (t := alive(3)) is not None:
                    w_acat(t)
                if (t := alive(5)) is not None:
                    w_pvb(t)
                    w_y2d(t)
                if (t := alive(6)) is not None:
                    w_T3mm(t)
                if (t := alive(9)) is not None:
                    w_H1mm(t)
                    w_h1s(t)
                if (t := alive(4)) is not None:
                    w_ycat0(t)
                if (t := alive(10)) is not None:
                    w_sq(t)
                if (t := alive(11)) is not None:
                    w_nsq(t)
                if (t := alive(2)) is not None:
                    w_apmm(t)
                if (t := alive(1)) is not None:
                    w_hpmm(t)
                    w_silu(t)

            # ============== gate tail (second sqrt + both tanhs) ==============
            # sqrt set is already loaded (first sqrt ran under the phase A
            # tail); one switch back to a tanh set covers both tanh calls
            # and all of phase B (Identity is filler everywhere).  Bridge
            # the remaining ACT-serial bubble with warm matmuls.
            warm(26)
            nc.scalar.activation(rt_b[:], nsq_ps[32:32 + hnt, :], AF.Sqrt,
                                 bias=zero_b[:])
            # sigmoid(norm + eps) = 0.5 tanh(0.5 norm + eps/2) + 0.5
            nc.scalar.activation(t9a[0:hnt, :], rt_a[:], AF.Tanh, bias=tanh_b[:], scale=0.5)
            nc.scalar.activation(t9b[0:hnt, :], rt_b[:], AF.Tanh, bias=tanh_b[:], scale=0.5)

            # ================= phase B (software-pipelined) =================
            trps = {}

            def emit_trp(t):
                trp = ps_pool.tile([H, BT], F32, name="trp", tag="ps")
                nc.tensor.matmul(
                    trp[:], esig_t[:, bass.ts(t, H)],
                    t9a[:] if t < hnt else t9b[:],
                    start=True, stop=True,
                )
                trps[t] = trp

            for t in range(min(4, nt)):
                emit_trp(t)
            outg = None
            for t in range(nt):
                g, r = divmod(t, 2)
                a1g = a1g_pool.tile([H, BT], BF16, name="a1g")
                nc.vector.tensor_tensor(
                    a1g[:], h1s_tiles[t][:], trps.pop(t)[:], mybir.AluOpType.mult
                )
                if r == 0:
                    outg = pv_pool.tile([H, BT], F32, name="outg", tag="pv")
                nc.tensor.matmul(
                    outg[32 * r:32 * (r + 1), :], ct["LT_W2"][:], a1g[:],
                    start=True, stop=True, skip_group_check=True,
                )
                if t + 4 < nt:
                    emit_trp(t + 4)
                if r == 1:
                    outs = outs_pool.tile([2 * D, BT], F32, name="outs")
                    nc.scalar.activation(outs[:], outg[0:2 * D, :], AF.Identity, bias=b22_t[:])
                    nc.gpsimd.dma_start(outT[:, bass.ts(g, BT)], outs[:])

    if split_waits:
        _split_multi_waits(nc)
    return nc


def _host_params(G, W_se1, b_se1, W_se2, b_se2, W1, b1, W2, b2, nt):
    import ml_dtypes
    f = np.float32
    bf = ml_dtypes.bfloat16
    G = np.asarray(G, f)
    Gflat = np.transpose(G, (0, 2, 1)).reshape(NG * D, D)  # [(g,i), j] = G[g,j,i]
    W1G = Gflat @ np.asarray(W1, f).T                      # [(g,i), m]
    hnt = nt // 2
    e_sig = np.zeros((hnt + 1, nt * H), f)
    for t in range(nt):
        e_sig[t % hnt, t * H:(t + 1) * H] = 0.5
    e_sig[hnt, :] = 0.5
    onsq2 = np.zeros((H, nt * hnt), f)
    for t in range(nt):
        onsq2[:, t * hnt + (t % hnt)] = 1.0
    w = {
        "LT_h": np.asarray(W_se1, f).T,
        "LT_At": np.repeat(np.asarray(W_se2, f).T[:, 0:4], 32, axis=1),
        "LT_Ab": np.repeat(np.asarray(W_se2, f).T[:, 4:8], 32, axis=1),
        "Bse1": np.asarray(b_se1, f).reshape(H, 1),
        "Bse2t": np.repeat(np.asarray(b_se2, f)[0:4], 32).reshape(H, 1),
        "Bse2b": np.repeat(np.asarray(b_se2, f)[4:8], 32).reshape(H, 1),
        "LT_z": np.tile(np.eye(D, dtype=f), (1, 4)),
        "LT_W1z": np.asarray(W1, f).T,
        "B1": np.asarray(b1, f).reshape(H, 1),
        "LT_W2": np.asarray(W2, f).T,
        "LT_t1": np.ascontiguousarray(W1G[:H]),
        "LT_b1k": np.ascontiguousarray(W1G[H:]),
    }
    for k, tname, bname in ((4, "LT_t4", "LT_b4"), (3, "LT_t3", "LT_b3"), (2, "LT_t2", "LT_b2k")):
        scaled = np.tile(Gflat * f(1.0 / k), (1, 4))
        w[tname] = np.ascontiguousarray(scaled[:H])
        w[bname] = np.ascontiguousarray(scaled[H:])
    p = {
        "CAT128": np.concatenate([w[n] for n, _ in _CAT128], axis=1).astype(bf),
        "CAT32": np.concatenate([w[n] for n, _ in _CAT32], axis=1).astype(bf),
        "CATB": np.concatenate([w[n] for n, _ in _CATB], axis=1).astype(f),
        "ONSQ": onsq2.astype(bf),
        "E_sig": e_sig.astype(bf),
        "B2_2": np.tile(np.asarray(b2, f), 2).reshape(2 * D, 1).astype(f),
    }
    return {k: np.ascontiguousarray(v) for k, v in p.items()}


def _run(z, G, W_se1, b_se1, W_se2, b_se2, W1, b1, W2, b2, trace=False, **trace_kw):
    import ml_dtypes
    z = np.asarray(z, np.float32)
    nt = BC // BT
    params = _host_params(G, W_se1, b_se1, W_se2, b_se2, W1, b1, W2, b2, nt)

    # shard: per-core feature-major bf16 slices
    zT = np.ascontiguousarray(
        z.reshape(NCORES, BC, D).transpose(0, 2, 1).astype(ml_dtypes.bfloat16)
    )

    nc = _build_program(BC)
    in_maps = [{"zT": zT[c], **params} for c in range(NCORES)]
    res = run_bass_kernel_spmd(nc, in_maps, list(range(NCORES)), trace=trace, **trace_kw)

    # outT[32r+d, g*BT+b] = out[(2g+r)*BT + b, d] per core
    outT = np.stack([res.results[c]["outT"] for c in range(NCORES)])
    out = (
        outT.reshape(NCORES, 2, D, nt // 2, BT)
        .transpose(0, 3, 1, 4, 2)
        .reshape(B, D)
    )
    return np.ascontiguousarray(out.astype(np.float32)), res


def kernel(z, G, W_se1, b_se1, W_se2, b_se2, W1, b1, W2, b2):
    out, _ = _run(z, G, W_se1, b_se1, W_se2, b_se2, W1, b1, W2, b2, trace=False)
    return out


if __name__ == "__main__":
    rng = np.random.default_rng(0)
    inputs = {
        "z": rng.standard_normal((B, D), dtype=np.float32),
        "G": (rng.standard_normal((NG, D, D)) * 0.1).astype(np.float32),
        "W_se1": (rng.standard_normal((H, D)) / np.sqrt(D)).astype(np.float32),
        "b_se1": np.zeros(H, np.float32),
        "W_se2": (rng.standard_normal((NG, H)) / np.sqrt(H)).astype(np.float32),
        "b_se2": np.zeros(NG, np.float32),
        "W1": (rng.standard_normal((H, D)) * 0.01).astype(np.float32),
        "b1": np.zeros(H, np.float32),
        "W2": (rng.standard_normal((D, H)) * 0.01).astype(np.float32),
        "b2": np.zeros(D, np.float32),
    }
    out = kernel(**inputs)
    print("kernel output", out.shape, out.dtype, float(np.abs(out).max()))


# revision 33
# speedup vs baseline: 1.4841x; 1.0599x over previous
"""EquivariantEvolution kernel for 8 Trainium2 NeuronCores (Bass/Tile).

Math (per sample, reference):
    alpha = Linear2(silu(Linear1(z)))                     # [NG]
    A     = sum_g alpha_g G_g                             # [D, D]
    z_t   = (I + A + A^2/2 + A^3/6 + A^4/24) z            # order-4 Taylor
    h1    = W1 z_t + b1
    out   = W2 (sigmoid(|h1| + eps) * h1) + b2

Device strategy (pure data-parallel over batch, feature-major layout):
  * Host pre-transposes z to [D, B/8] bf16 per core; weights are reshaped
    on host so the device runs only matmuls / elementwise ops, all in bf16
    (PSUM accumulation stays fp32).
  * Horner:  v <- z + (1/k) A v.  A v is one K=128 contraction:
      y[(g,i), b] = alpha_g[b] * v[i, b]   (elementwise; alpha replicated
      across the 32 i-partitions by construction)
      (A v)[j, b] = sum_{(g,i)} G[g,j,i] y[(g,i),b]   (two K=128 matmuls)
    lhsT is tiled 4x along M so the output lands pre-replicated for the
    next step's elementwise multiply; the z-add is a third K=32 matmul.
  * Engine balance per tile: PE 16 matmul slots; DVE gets the two
    PSUM-sourced taylor multiplies plus one 2x-rate SBUF pair; ACT runs
    silu/alpha-bias/h1/one pv->bf16 staging copy; GpSimd runs the z-based
    multiply and the h1^2 square; Sync carries input DMA, GpSimd queue the
    output DMA.
  * HAM discipline: zero-weight matmul bursts open the PE clock gate
    (K=8/8) at startup and across the sqrt/tanh ACT-table-switch bubble;
    everything else is a dense bf16 matmul stream, so the gate stays open.
  * Gate: sigmoid(norm + eps) = 0.5 tanh(norm/2 + eps/2) + 0.5 is produced
    broadcast to [H, BT] by a single K=17 matmul (16 tanh rows + a
    constant-ones row carrying the +0.5).  Phase B is software-pipelined:
    gate matmul, DVE multiply, one output matmul per tile, outputs of 2
    tiles packed per PSUM bank before a single bias+store pass.
"""

import os
import sys

import numpy as np

for _p in ("/opt/trn_rl_repo", "/root/.axon_site/_ro/trn_rl_repo"):
    if os.path.isdir(_p) and _p not in sys.path:
        sys.path.insert(0, _p)

import concourse.bass as bass
import concourse.mybir as mybir
import concourse.tile as tile
from concourse.bass_utils import run_bass_kernel_spmd

B, D, H, NG = 65536, 32, 128, 8
NCORES = 8
BC = B // NCORES          # samples per core
BT = 512                  # samples (free-dim columns) per tile
EPS = 1e-6
F32 = mybir.dt.float32
BF16 = mybir.dt.bfloat16
AF = mybir.ActivationFunctionType

# bf16 weight blob layouts: name -> (col offset, n cols); all [128, .] in
# CAT128, all [32, .] in CAT32.  Order matters only for the host packer.
_CAT128 = [
    ("LT_At", H), ("LT_Ab", H),
    ("LT_t4", H), ("LT_b4", H), ("LT_t3", H), ("LT_b3", H),
    ("LT_t2", H), ("LT_b2k", H), ("LT_t1", H), ("LT_b1k", H),
    ("LT_W2", D),
]
_CAT32 = [("LT_h", H), ("LT_z", H), ("LT_W1z", H)]
_CATB = [("Bse1", 1), ("Bse2t", 1), ("Bse2b", 1), ("B1", 1)]  # [128,1] f32


def _cat_cols(cat):
    off, out = 0, {}
    for name, w in cat:
        out[name] = (off, w)
        off += w
    return out, off


def _split_multi_waits(nc, max_waits=1):
    """This toolchain's walrus rejects >1 sync-wait on an instruction
    ("Too many sync wait commands"); hoist extra waits onto preceding
    same-engine NOPs (in-order engines make this semantics-preserving)."""
    n_new = 0
    for f in nc.m.functions:
        for bb in f.blocks:
            out = []
            for ins in bb.instructions:
                si = getattr(ins, "sync_info", None)
                if si is not None and si.on_wait and len(si.on_wait) > max_waits:
                    waits = list(si.on_wait)
                    chunks = [waits[i:i + max_waits] for i in range(0, len(waits), max_waits)]
                    for ci, ch in enumerate(chunks[:-1]):
                        nop = mybir.InstNoOp(
                            name=f"{ins.name}-wsplit{ci}",
                            engine=ins.engine,
                            sync_info=mybir.SyncInfo(on_wait=ch, on_update=[]),
                            bass_nofuse=True,
                        )
                        out.append(nop)
                        n_new += 1
                    ins.sync_info = mybir.SyncInfo(on_wait=chunks[-1], on_update=si.on_update)
                out.append(ins)
            bb.instructions[:] = out
    return n_new


def _build_program(bc: int, sim_safe: bool = False, split_waits: bool = True):
    """Trace the per-core Bass program for bc samples.

    sim_safe decomposes Silu into Sigmoid*x (CoreSim has no Silu handler);
    the hardware path uses the native Silu LUT.
    """
    nt = bc // BT
    ng2 = nt // 2             # output 2-tile groups
    nc = bass.Bass()

    c128_cols, c128_w = _cat_cols(_CAT128)
    c32_cols, c32_w = _cat_cols(_CAT32)
    cb_cols, cb_w = _cat_cols(_CATB)

    zT = nc.declare_dram_parameter("zT", [D, bc], BF16, isOutput=False)
    cat128 = nc.declare_dram_parameter("CAT128", [H, c128_w], BF16, isOutput=False)
    cat32 = nc.declare_dram_parameter("CAT32", [D, c32_w], BF16, isOutput=False)
    catb = nc.declare_dram_parameter("CATB", [H, cb_w], F32, isOutput=False)
    hnt = nt // 2
    onsq = nc.declare_dram_parameter("ONSQ", [H, nt * hnt], BF16, isOutput=False)
    esig = nc.declare_dram_parameter("E_sig", [hnt + 1, nt * H], BF16, isOutput=False)
    b22 = nc.declare_dram_parameter("B2_2", [2 * D, 1], F32, isOutput=False)
    outT = nc.declare_dram_parameter("outT", [2 * D, ng2 * BT], F32, isOutput=True)

    with tile.TileContext(nc) as tc:
        with (
            tc.tile_pool(name="consts", bufs=1) as consts,
            tc.tile_pool(name="zv4", bufs=7) as zv4_pool,
            tc.tile_pool(name="hs", bufs=4) as hs_pool,
            tc.tile_pool(name="acat", bufs=8) as acat_pool,
            tc.tile_pool(name="ycat", bufs=12) as ycat_pool,
            tc.tile_pool(name="pvb", bufs=4) as pvb_pool,
            tc.tile_pool(name="sq", bufs=4) as sq_pool,
            tc.tile_pool(name="h1s", bufs=nt) as h1s_pool,
            tc.tile_pool(name="gate", bufs=1) as gate_pool,
            tc.tile_pool(name="a1g", bufs=4) as a1g_pool,
            tc.tile_pool(name="outs", bufs=4) as outs_pool,
            tc.tile_pool(name="ps", bufs=3, space=bass.MemorySpace.PSUM) as ps_pool,
            tc.tile_pool(name="pv", bufs=4, space=bass.MemorySpace.PSUM) as pv_pool,
            tc.tile_pool(name="psn", bufs=1, space=bass.MemorySpace.PSUM) as psn_pool,
        ):
            # ---- HAM trigger: zero-weight matmuls need no DMA'd data, so
            # the PE clock gate opens while constants stream in.  Warm
            # bursts borrow a pv-pool bank (freed back to rotation). ----
            wscr = consts.tile([H, BT], BF16, name="wscr")
            nc.vector.memset(wscr[:], 0.0)

            def warm(n, cols=BT):
                wps = pv_pool.tile([H, BT], F32, name="wps", tag="pv")
                for _ in range(n):
                    nc.tensor.matmul(wps[:, 0:cols], wscr[:, 0:H], wscr[:, 0:cols],
                                     start=True, stop=True)

            warm(16)

            # ---- load constants into SBUF (few large DMAs) ----
            c128_t = consts.tile([H, c128_w], BF16, name="c_cat128")
            nc.sync.dma_start(c128_t[:], cat128[:])
            c32_t = consts.tile([D, c32_w], BF16, name="c_cat32")
            nc.sync.dma_start(c32_t[:], cat32[:])
            cb_t = consts.tile([H, cb_w], F32, name="c_catb")
            nc.sync.dma_start(cb_t[:], catb[:])
            onsq_t = consts.tile([H, nt * hnt], BF16, name="c_onsq")
            nc.sync.dma_start(onsq_t[:], onsq[:])
            esig_t = consts.tile([hnt + 1, nt * H], BF16, name="c_esig")
            nc.sync.dma_start(esig_t[:], esig[:])
            b22_t = consts.tile([2 * D, 1], F32, name="c_b22")
            nc.sync.dma_start(b22_t[:], b22[:])

            ct = {}
            for name, (off, w) in c128_cols.items():
                ct[name] = c128_t[:, off:off + w]
            for name, (off, w) in c32_cols.items():
                ct[name] = c32_t[:, off:off + w]
            for name, (off, w) in cb_cols.items():
                ct[name] = cb_t[:, off:off + w]

            zero_b = consts.tile([hnt, 1], F32, name="zero_b")
            nc.vector.memset(zero_b[:], 0.0)
            tanh_b = consts.tile([hnt, 1], F32, name="tanh_b")
            nc.vector.memset(tanh_b[:], 0.5 * EPS)
            # per-half-gate tanh rows 0..7 + constant-ones row 8 (the +0.5
            # path); memset whole tiles, tanh later overwrites rows 0..7
            t9a = consts.tile([hnt + 1, BT], BF16, name="t9a")
            nc.vector.memset(t9a[:], 1.0)
            t9b = consts.tile([hnt + 1, BT], BF16, name="t9b")
            nc.vector.memset(t9b[:], 1.0)

            taylor = [
                (ct["LT_t4"], ct["LT_b4"]),
                (ct["LT_t3"], ct["LT_b3"]),
                (ct["LT_t2"], ct["LT_b2k"]),
            ]

            h1s_tiles = [None] * nt
            # two independent accumulation groups in one bank: tiles 0..7 at
            # base partition 0, tiles 8..15 at base 32 (matmul output base
            # partitions must be 0/32/64), so the first sqrt can run while
            # the second half of phase A is still streaming
            nsq_ps = psn_pool.tile([40, BT], F32, name="nsq_ps", tag="nsq")

            # ================= phase A: wavefront schedule =================
            # The per-tile chain is split into 11 macro-stages; tile t runs
            # stage m at wave t+m, so at steady state every wave carries one
            # tile in each stage.  Ops are emitted per wave in per-engine
            # readiness order, which makes each in-order engine queue process
            # ops exactly as their inputs become available: PE ~16 matmul
            # slots, ACT 5 ops, DVE 4 ops, GpSimd 2 ops per wave -- all four
            # engines near their ~3.4us/tile budget with no cross-pair
            # serialization.
            st = {t: {} for t in range(nt)}  # per-tile in-flight tensors

            def mm_T(t, lt_top, lt_bot, key_in, with_z):
                s = st[t]
                pv = pv_pool.tile([H, BT], F32, name="pv", tag="pv")
                ycat = s[key_in]
                nc.tensor.matmul(pv[:], lt_top[:], ycat[:, 0, :], start=True, stop=False)
                nc.tensor.matmul(pv[:], lt_bot[:], ycat[:, 1, :], start=False,
                                 stop=not with_z)
                if with_z:
                    nc.tensor.matmul(pv[:], ct["LT_z"][:], s["z"][0:D, :],
                                     start=False, stop=True)
                return pv

            def w_T2mm(t):
                st[t]["pv2"] = mm_T(t, ct["LT_t2"], ct["LT_b2k"], "y3", True)

            def w_ybc(t, key_pv, key_out):
                s = st[t]
                ycat = ycat_pool.tile([H, 2, BT], BF16, name="ycat")
                nc.vector.tensor_tensor(
                    ycat[:], s["acat"][:],
                    s.pop(key_pv)[:, None, :].broadcast_to([H, 2, BT]),
                    mybir.AluOpType.mult,
                )
                s[key_out] = ycat

            def w_T4mm(t):
                st[t]["pv4"] = mm_T(t, ct["LT_t4"], ct["LT_b4"], "y0", True)

            def w_acat(t):
                s = st[t]
                acat = acat_pool.tile([H, 2, BT], BF16, name="acat")
                nc.scalar.activation(acat[:, 0, :], s.pop("apt")[:], AF.Identity,
                                     bias=ct["Bse2t"][:])
                nc.scalar.activation(acat[:, 1, :], s.pop("apb")[:], AF.Identity,
                                     bias=ct["Bse2b"][:])
                s["acat"] = acat

            def w_pvb(t):
                s = st[t]
                pvb = pvb_pool.tile([H, BT], BF16, name="pvb")
                nc.scalar.activation(pvb[:], s.pop("pv4")[:], AF.Identity)
                s["pvb"] = pvb

            def w_y2d(t):
                # all-bf16 SBUF multiply: one op, broadcast pvb over halves
                s = st[t]
                pvb = s.pop("pvb")
                ycat = ycat_pool.tile([H, 2, BT], BF16, name="ycat")
                nc.vector.tensor_tensor(
                    ycat[:], s["acat"][:],
                    pvb[:, None, :].broadcast_to([H, 2, BT]),
                    mybir.AluOpType.mult,
                )
                s["y4"] = ycat

            def w_T3mm(t):
                st[t]["pv3"] = mm_T(t, ct["LT_t3"], ct["LT_b3"], "y4", True)

            def w_H1mm(t):
                s = st[t]
                h1p = pv_pool.tile([H, BT], F32, name="h1p", tag="pv")
                ycat = s.pop("y2")
                nc.tensor.matmul(h1p[:], ct["LT_t1"][:], ycat[:, 0, :], start=True, stop=False)
                nc.tensor.matmul(h1p[:], ct["LT_b1k"][:], ycat[:, 1, :], start=False, stop=False)
                nc.tensor.matmul(h1p[:], ct["LT_W1z"][:], s["z"][0:D, :],
                                 start=False, stop=True)
                s["h1p"] = h1p
                s.pop("acat")

            def w_h1s(t):
                s = st[t]
                h1s = h1s_pool.tile([H, BT], BF16, name="h1s")
                nc.scalar.activation(h1s[:], s.pop("h1p")[:], AF.Identity, bias=ct["B1"][:])
                h1s_tiles[t] = h1s

            def w_sq(t):
                # square on GpSimd from the bf16 h1 (both operands SBUF)
                sq = sq_pool.tile([H, BT], BF16, name="sq")
                nc.gpsimd.tensor_tensor(sq[:], h1s_tiles[t][:], h1s_tiles[t][:],
                                        mybir.AluOpType.mult)
                st[t]["sq"] = sq

            def w_nsq(t):
                out = nsq_ps[0:hnt, :] if t < hnt else nsq_ps[32:32 + hnt, :]
                nc.tensor.matmul(
                    out, onsq_t[:, bass.ts(t, hnt)], st[t].pop("sq")[:],
                    start=(t % hnt == 0), stop=(t % hnt == hnt - 1),
                    skip_group_check=True,
                )

            def w_apmm(t):
                s = st[t]
                apt = ps_pool.tile([H, BT], F32, name="apt", tag="ps")
                nc.tensor.matmul(apt[:], ct["LT_At"][:], s["hs"][:], start=True, stop=True)
                apb = ps_pool.tile([H, BT], F32, name="apb", tag="ps")
                nc.tensor.matmul(apb[:], ct["LT_Ab"][:], s.pop("hs")[:], start=True, stop=True)
                s["apt"], s["apb"] = apt, apb

            def w_hpmm(t):
                s = st[t]
                hp = ps_pool.tile([H, BT], F32, name="hp", tag="ps")
                nc.tensor.matmul(hp[:], ct["LT_h"][:], s["z"][0:D, :], start=True, stop=True)
                s["hp"] = hp

            def w_silu(t):
                s = st[t]
                hp = s.pop("hp")
                hs = hs_pool.tile([H, BT], BF16, name="hs")
                if sim_safe:
                    sg = hs_pool.tile([H, BT], F32, name="sg")
                    nc.scalar.activation(sg[:], hp[:], AF.Sigmoid, bias=ct["Bse1"][:])
                    hx = hs_pool.tile([H, BT], F32, name="hx")
                    nc.scalar.activation(hx[:], hp[:], AF.Identity, bias=ct["Bse1"][:])
                    nc.vector.tensor_tensor(hs[:], sg[:], hx[:], mybir.AluOpType.mult)
                else:
                    nc.scalar.activation(hs[:], hp[:], AF.Silu, bias=ct["Bse1"][:])
                s["hs"] = hs

            def w_ycat0(t):
                # all-SBUF bf16 multiply: runs on GpSimd to keep DVE free
                # for the PSUM-sourced taylor multiplies
                s = st[t]
                ycat = ycat_pool.tile([H, 2, BT], BF16, name="ycat0")
                nc.gpsimd.tensor_tensor(
                    ycat[:], s["acat"][:],
                    s["z"][:, None, :].broadcast_to([H, 2, BT]),
                    mybir.AluOpType.mult,
                )
                s["y0"] = ycat

            def w_dma(t):
                # pair-shared input load (4 replica DMAs per 2 tiles)
                zv4 = zv4_pool.tile([H, 2 * BT], BF16, name="zv4")
                for r in range(4):
                    nc.sync.dma_start(
                        zv4[32 * r:32 * (r + 1), :],
                        zT[:, bass.ts(t // 2, 2 * BT)],
                    )
                st[t]["z"] = zv4[:, 0:BT]
                st[t + 1]["z"] = zv4[:, BT:2 * BT]

            # wave w, tile w-m runs macro-stage m:
            #  m0 dma | m1 hp,silu | m2 ap | m3 acat | m4 ycat0
            #  m5 T4,pvb,y2d | m6 T3 | m7 ybc(pv3) | m8 T2,ybc(pv2)
            #  m9 H1,h1s | m10 sq | m11 nsq
            def alive(m):
                t = w - m
                return t if 0 <= t < nt else None

            rt_a = gate_pool.tile([hnt, BT], F32, name="rt_a")
            rt_b = gate_pool.tile([hnt, BT], F32, name="rt_b")

            for w in range(nt + 12):
                if 1 <= w <= 9:
                    warm(3)
                if w == nt + 5:
                    # first-half norms closed (wave nt+3); the switch to the
                    # sqrt table and this sqrt hide under the phase A tail --
                    # the only remaining phase A ACT ops are Identity, which
                    # is filler in every table set
                    nc.scalar.activation(rt_a[:], nsq_ps[0:hnt, :], AF.Sqrt,
                                         bias=zero_b[:])
                if (t := alive(0)) is not None and t % 2 == 0:
                    w_dma(t)
                if (t := alive(8)) is not None:
                    w_T2mm(t)
                if (t := alive(7)) is not None:
                    w_ybc(t, "pv3", "y3")
                if (t := alive(8)) is not None:
                    w_ybc(t, "pv2", "y2")
                if (t := alive(5)) is not None:
                    w_T4mm(t)
                if (t := alive(3)) is not None:
                    w_acat(t)
                if (t := alive(5)) is not None:
                    w_pvb(t)
                    w_y2d(t)
                if (t := alive(6)) is not None:
                    w_T3mm(t)
                if (t := alive(9)) is not None:
                    w_H1mm(t)
                    w_h1s(t)
                if (t := alive(4)) is not None:
                    w_ycat0(t)
                if (t := alive(10)) is not None:
                    w_sq(t)
                if (t := alive(11)) is not None:
                    w_nsq(t)
                if (t := alive(2)) is not None:
                    w_apmm(t)
                if (t := alive(1)) is not None:
                    w_hpmm(t)
                    w_silu(t)

            # ============== gate tail (second sqrt + both tanhs) ==============
            # sqrt set is already loaded (first sqrt ran under the phase A
            # tail); one switch back to a tanh set covers both tanh calls
            # and all of phase B (Identity is filler everywhere).  Bridge
            # the remaining ACT-serial bubble (~2.5us) with warm matmuls --
            # no more, or they delay the gate matmuls in the PE queue.
            warm(8)
            nc.scalar.activation(rt_b[:], nsq_ps[32:32 + hnt, :], AF.Sqrt,
                                 bias=zero_b[:])
            # sigmoid(norm + eps) = 0.5 tanh(0.5 norm + eps/2) + 0.5
            nc.scalar.activation(t9a[0:hnt, :], rt_a[:], AF.Tanh, bias=tanh_b[:], scale=0.5)
            nc.scalar.activation(t9b[0:hnt, :], rt_b[:], AF.Tanh, bias=tanh_b[:], scale=0.5)

            # ================= phase B (software-pipelined) =================
            trps = {}

            def emit_trp(t):
                trp = ps_pool.tile([H, BT], F32, name="trp", tag="ps")
                nc.tensor.matmul(
                    trp[:], esig_t[:, bass.ts(t, H)],
                    t9a[:] if t < hnt else t9b[:],
                    start=True, stop=True,
                )
                trps[t] = trp

            for t in range(min(4, nt)):
                emit_trp(t)
            outg = None
            for t in range(nt):
                g, r = divmod(t, 2)
                a1g = a1g_pool.tile([H, BT], BF16, name="a1g")
                nc.vector.tensor_tensor(
                    a1g[:], h1s_tiles[t][:], trps.pop(t)[:], mybir.AluOpType.mult
                )
                if r == 0:
                    outg = pv_pool.tile([H, BT], F32, name="outg", tag="pv")
                nc.tensor.matmul(
                    outg[32 * r:32 * (r + 1), :], ct["LT_W2"][:], a1g[:],
                    start=True, stop=True, skip_group_check=True,
                )
                if t + 4 < nt:
                    emit_trp(t + 4)
                if r == 1:
                    outs = outs_pool.tile([2 * D, BT], F32, name="outs")
                    nc.scalar.activation(outs[:], outg[0:2 * D, :], AF.Identity, bias=b22_t[:])
                    # alternate queues so the final stores drain in parallel
                    eng = nc.gpsimd if g % 2 == 0 else nc.sync
                    eng.dma_start(outT[:, bass.ts(g, BT)], outs[:])

    if split_waits:
        _split_multi_waits(nc)
    return nc


def _host_params(G, W_se1, b_se1, W_se2, b_se2, W1, b1, W2, b2, nt):
    import ml_dtypes
    f = np.float32
    bf = ml_dtypes.bfloat16
    G = np.asarray(G, f)
    Gflat = np.transpose(G, (0, 2, 1)).reshape(NG * D, D)  # [(g,i), j] = G[g,j,i]
    W1G = Gflat @ np.asarray(W1, f).T                      # [(g,i), m]
    hnt = nt // 2
    e_sig = np.zeros((hnt + 1, nt * H), f)
    for t in range(nt):
        e_sig[t % hnt, t * H:(t + 1) * H] = 0.5
    e_sig[hnt, :] = 0.5
    onsq2 = np.zeros((H, nt * hnt), f)
    for t in range(nt):
        onsq2[:, t * hnt + (t % hnt)] = 1.0
    w = {
        "LT_h": np.asarray(W_se1, f).T,
        "LT_At": np.repeat(np.asarray(W_se2, f).T[:, 0:4], 32, axis=1),
        "LT_Ab": np.repeat(np.asarray(W_se2, f).T[:, 4:8], 32, axis=1),
        "Bse1": np.asarray(b_se1, f).reshape(H, 1),
        "Bse2t": np.repeat(np.asarray(b_se2, f)[0:4], 32).reshape(H, 1),
        "Bse2b": np.repeat(np.asarray(b_se2, f)[4:8], 32).reshape(H, 1),
        "LT_z": np.tile(np.eye(D, dtype=f), (1, 4)),
        "LT_W1z": np.asarray(W1, f).T,
        "B1": np.asarray(b1, f).reshape(H, 1),
        "LT_W2": np.asarray(W2, f).T,
        "LT_t1": np.ascontiguousarray(W1G[:H]),
        "LT_b1k": np.ascontiguousarray(W1G[H:]),
    }
    for k, tname, bname in ((4, "LT_t4", "LT_b4"), (3, "LT_t3", "LT_b3"), (2, "LT_t2", "LT_b2k")):
        scaled = np.tile(Gflat * f(1.0 / k), (1, 4))
        w[tname] = np.ascontiguousarray(scaled[:H])
        w[bname] = np.ascontiguousarray(scaled[H:])
    p = {
        "CAT128": np.concatenate([w[n] for n, _ in _CAT128], axis=1).astype(bf),
        "CAT32": np.concatenate([w[n] for n, _ in _CAT32], axis=1).astype(bf),
        "CATB": np.concatenate([w[n] for n, _ in _CATB], axis=1).astype(f),
        "ONSQ": onsq2.astype(bf),
        "E_sig": e_sig.astype(bf),
        "B2_2": np.tile(np.asarray(b2, f), 2).reshape(2 * D, 1).astype(f),
    }
    return {k: np.ascontiguousarray(v) for k, v in p.items()}


def _run(z, G, W_se1, b_se1, W_se2, b_se2, W1, b1, W2, b2, trace=False, **trace_kw):
    import ml_dtypes
    z = np.asarray(z, np.float32)
    nt = BC // BT
    params = _host_params(G, W_se1, b_se1, W_se2, b_se2, W1, b1, W2, b2, nt)

    # shard: per-core feature-major bf16 slices
    zT = np.ascontiguousarray(
        z.reshape(NCORES, BC, D).transpose(0, 2, 1).astype(ml_dtypes.bfloat16)
    )

    nc = _build_program(BC)
    in_maps = [{"zT": zT[c], **params} for c in range(NCORES)]
    res = run_bass_kernel_spmd(nc, in_maps, list(range(NCORES)), trace=trace, **trace_kw)

    # outT[32r+d, g*BT+b] = out[(2g+r)*BT + b, d] per core
    outT = np.stack([res.results[c]["outT"] for c in range(NCORES)])
    out = (
        outT.reshape(NCORES, 2, D, nt // 2, BT)
        .transpose(0, 3, 1, 4, 2)
        .reshape(B, D)
    )
    return np.ascontiguousarray(out.astype(np.float32)), res


def kernel(z, G, W_se1, b_se1, W_se2, b_se2, W1, b1, W2, b2):
    out, _ = _run(z, G, W_se1, b_se1, W_se2, b_se2, W1, b1, W2, b2, trace=False)
    return out


if __name__ == "__main__":
    rng = np.random.default_rng(0)
    inputs = {
        "z": rng.standard_normal((B, D), dtype=np.float32),
        "G": (rng.standard_normal((NG, D, D)) * 0.1).astype(np.float32),
        "W_se1": (rng.standard_normal((H, D)) / np.sqrt(D)).astype(np.float32),
        "b_se1": np.zeros(H, np.float32),
        "W_se2": (rng.standard_normal((NG, H)) / np.sqrt(H)).astype(np.float32),
        "b_se2": np.zeros(NG, np.float32),
        "W1": (rng.standard_normal((H, D)) * 0.01).astype(np.float32),
        "b1": np.zeros(H, np.float32),
        "W2": (rng.standard_normal((D, H)) * 0.01).astype(np.float32),
        "b2": np.zeros(D, np.float32),
    }
    out = kernel(**inputs)
    print("kernel output", out.shape, out.dtype, float(np.abs(out).max()))


# revision 35
# speedup vs baseline: 1.4861x; 1.0013x over previous
"""EquivariantEvolution kernel for 8 Trainium2 NeuronCores (Bass/Tile).

Math (per sample, reference):
    alpha = Linear2(silu(Linear1(z)))                     # [NG]
    A     = sum_g alpha_g G_g                             # [D, D]
    z_t   = (I + A + A^2/2 + A^3/6 + A^4/24) z            # order-4 Taylor
    h1    = W1 z_t + b1
    out   = W2 (sigmoid(|h1| + eps) * h1) + b2

Device strategy (pure data-parallel over batch, feature-major layout):
  * Host pre-transposes z to [D, B/8] bf16 per core; weights are reshaped
    on host so the device runs only matmuls / elementwise ops, all in bf16
    (PSUM accumulation stays fp32).
  * Horner:  v <- z + (1/k) A v.  A v is one K=128 contraction:
      y[(g,i), b] = alpha_g[b] * v[i, b]   (elementwise; alpha replicated
      across the 32 i-partitions by construction)
      (A v)[j, b] = sum_{(g,i)} G[g,j,i] y[(g,i),b]   (two K=128 matmuls)
    lhsT is tiled 4x along M so the output lands pre-replicated for the
    next step's elementwise multiply; the z-add is a third K=32 matmul.
  * Engine balance per tile: PE 16 matmul slots; DVE gets the two
    PSUM-sourced taylor multiplies plus one 2x-rate SBUF pair; ACT runs
    silu/alpha-bias/h1/one pv->bf16 staging copy; GpSimd runs the z-based
    multiply and the h1^2 square; Sync carries input DMA, GpSimd queue the
    output DMA.
  * HAM discipline: zero-weight matmul bursts open the PE clock gate
    (K=8/8) at startup and across the sqrt/tanh ACT-table-switch bubble;
    everything else is a dense bf16 matmul stream, so the gate stays open.
  * Gate: sigmoid(norm + eps) = 0.5 tanh(norm/2 + eps/2) + 0.5 is produced
    broadcast to [H, BT] by a single K=17 matmul (16 tanh rows + a
    constant-ones row carrying the +0.5).  Phase B is software-pipelined:
    gate matmul, DVE multiply, one output matmul per tile, outputs of 2
    tiles packed per PSUM bank before a single bias+store pass.
"""

import os
import sys

import numpy as np

for _p in ("/opt/trn_rl_repo", "/root/.axon_site/_ro/trn_rl_repo"):
    if os.path.isdir(_p) and _p not in sys.path:
        sys.path.insert(0, _p)

import concourse.bass as bass
import concourse.mybir as mybir
import concourse.tile as tile
from concourse.bass_utils import run_bass_kernel_spmd

B, D, H, NG = 65536, 32, 128, 8
NCORES = 8
BC = B // NCORES          # samples per core
BT = 512                  # samples (free-dim columns) per tile
EPS = 1e-6
F32 = mybir.dt.float32
BF16 = mybir.dt.bfloat16
AF = mybir.ActivationFunctionType

# bf16 weight blob layouts: name -> (col offset, n cols); all [128, .] in
# CAT128, all [32, .] in CAT32.  Order matters only for the host packer.
_CAT128 = [
    ("LT_At", H), ("LT_Ab", H),
    ("LT_t4", H), ("LT_b4", H), ("LT_t3", H), ("LT_b3", H),
    ("LT_t2", H), ("LT_b2k", H), ("LT_t1", H), ("LT_b1k", H),
    ("LT_W2", D),
]
_CAT32 = [("LT_h", H), ("LT_z", H), ("LT_W1z", H)]
_CATB = [("Bse1", 1), ("Bse2t", 1), ("Bse2b", 1), ("B1", 1)]  # [128,1] f32


def _cat_cols(cat):
    off, out = 0, {}
    for name, w in cat:
        out[name] = (off, w)
        off += w
    return out, off


def _split_multi_waits(nc, max_waits=1):
    """This toolchain's walrus rejects >1 sync-wait on an instruction
    ("Too many sync wait commands"); hoist extra waits onto preceding
    same-engine NOPs (in-order engines make this semantics-preserving)."""
    n_new = 0
    for f in nc.m.functions:
        for bb in f.blocks:
            out = []
            for ins in bb.instructions:
                si = getattr(ins, "sync_info", None)
                if si is not None and si.on_wait and len(si.on_wait) > max_waits:
                    waits = list(si.on_wait)
                    chunks = [waits[i:i + max_waits] for i in range(0, len(waits), max_waits)]
                    for ci, ch in enumerate(chunks[:-1]):
                        nop = mybir.InstNoOp(
                            name=f"{ins.name}-wsplit{ci}",
                            engine=ins.engine,
                            sync_info=mybir.SyncInfo(on_wait=ch, on_update=[]),
                            bass_nofuse=True,
                        )
                        out.append(nop)
                        n_new += 1
                    ins.sync_info = mybir.SyncInfo(on_wait=chunks[-1], on_update=si.on_update)
                out.append(ins)
            bb.instructions[:] = out
    return n_new


def _build_program(bc: int, sim_safe: bool = False, split_waits: bool = True):
    """Trace the per-core Bass program for bc samples.

    sim_safe decomposes Silu into Sigmoid*x (CoreSim has no Silu handler);
    the hardware path uses the native Silu LUT.
    """
    nt = bc // BT
    ng2 = nt // 2             # output 2-tile groups
    nc = bass.Bass()

    c128_cols, c128_w = _cat_cols(_CAT128)
    c32_cols, c32_w = _cat_cols(_CAT32)
    cb_cols, cb_w = _cat_cols(_CATB)

    zT = nc.declare_dram_parameter("zT", [D, bc], BF16, isOutput=False)
    cat128 = nc.declare_dram_parameter("CAT128", [H, c128_w], BF16, isOutput=False)
    cat32 = nc.declare_dram_parameter("CAT32", [D, c32_w], BF16, isOutput=False)
    catb = nc.declare_dram_parameter("CATB", [H, cb_w], F32, isOutput=False)
    hnt = nt // 2
    onsq = nc.declare_dram_parameter("ONSQ", [H, nt * hnt], BF16, isOutput=False)
    esig = nc.declare_dram_parameter("E_sig", [hnt + 1, nt * H], BF16, isOutput=False)
    b22 = nc.declare_dram_parameter("B2_2", [2 * D, 1], F32, isOutput=False)
    outT = nc.declare_dram_parameter("outT", [2 * D, ng2 * BT], F32, isOutput=True)

    with tile.TileContext(nc) as tc:
        with (
            tc.tile_pool(name="consts", bufs=1) as consts,
            tc.tile_pool(name="zv4", bufs=7) as zv4_pool,
            tc.tile_pool(name="hs", bufs=4) as hs_pool,
            tc.tile_pool(name="acat", bufs=8) as acat_pool,
            tc.tile_pool(name="ycat", bufs=12) as ycat_pool,
            tc.tile_pool(name="pvb", bufs=4) as pvb_pool,
            tc.tile_pool(name="sq", bufs=4) as sq_pool,
            tc.tile_pool(name="h1s", bufs=nt) as h1s_pool,
            tc.tile_pool(name="gate", bufs=1) as gate_pool,
            tc.tile_pool(name="a1g", bufs=4) as a1g_pool,
            tc.tile_pool(name="outs", bufs=4) as outs_pool,
            tc.tile_pool(name="ps", bufs=3, space=bass.MemorySpace.PSUM) as ps_pool,
            tc.tile_pool(name="pv", bufs=4, space=bass.MemorySpace.PSUM) as pv_pool,
            tc.tile_pool(name="psn", bufs=1, space=bass.MemorySpace.PSUM) as psn_pool,
        ):
            # ---- HAM trigger: zero-weight matmuls need no DMA'd data, so
            # the PE clock gate opens while constants stream in.  Warm
            # bursts borrow a pv-pool bank (freed back to rotation). ----
            wscr = consts.tile([H, BT], BF16, name="wscr")
            nc.vector.memset(wscr[:], 0.0)
            # preload the silu table set with a tiny dummy op so the ~2.7us
            # ACT_TABLE_LOAD overlaps the constant DMAs instead of sitting
            # on the first tile's critical chain
            sdum = consts.tile([8, 8], F32, name="sdum")
            nc.scalar.activation(sdum[:], wscr[0:8, 0:8], AF.Silu)

            def warm(n, cols=BT):
                wps = pv_pool.tile([H, BT], F32, name="wps", tag="pv")
                for _ in range(n):
                    nc.tensor.matmul(wps[:, 0:cols], wscr[:, 0:H], wscr[:, 0:cols],
                                     start=True, stop=True)

            warm(16)

            # ---- load constants into SBUF (few large DMAs; the small
            # first-needed extractor weights land first) ----
            c32_t = consts.tile([D, c32_w], BF16, name="c_cat32")
            nc.sync.dma_start(c32_t[:], cat32[:])
            c128_t = consts.tile([H, c128_w], BF16, name="c_cat128")
            nc.sync.dma_start(c128_t[:], cat128[:])
            cb_t = consts.tile([H, cb_w], F32, name="c_catb")
            nc.sync.dma_start(cb_t[:], catb[:])
            onsq_t = consts.tile([H, nt * hnt], BF16, name="c_onsq")
            nc.sync.dma_start(onsq_t[:], onsq[:])
            esig_t = consts.tile([hnt + 1, nt * H], BF16, name="c_esig")
            nc.sync.dma_start(esig_t[:], esig[:])
            b22_t = consts.tile([2 * D, 1], F32, name="c_b22")
            nc.sync.dma_start(b22_t[:], b22[:])

            ct = {}
            for name, (off, w) in c128_cols.items():
                ct[name] = c128_t[:, off:off + w]
            for name, (off, w) in c32_cols.items():
                ct[name] = c32_t[:, off:off + w]
            for name, (off, w) in cb_cols.items():
                ct[name] = cb_t[:, off:off + w]

            zero_b = consts.tile([hnt, 1], F32, name="zero_b")
            nc.vector.memset(zero_b[:], 0.0)
            tanh_b = consts.tile([hnt, 1], F32, name="tanh_b")
            nc.vector.memset(tanh_b[:], 0.5 * EPS)
            # per-half-gate tanh rows 0..7 + constant-ones row 8 (the +0.5
            # path); memset whole tiles, tanh later overwrites rows 0..7
            t9a = consts.tile([hnt + 1, BT], BF16, name="t9a")
            nc.vector.memset(t9a[:], 1.0)
            t9b = consts.tile([hnt + 1, BT], BF16, name="t9b")
            nc.vector.memset(t9b[:], 1.0)

            taylor = [
                (ct["LT_t4"], ct["LT_b4"]),
                (ct["LT_t3"], ct["LT_b3"]),
                (ct["LT_t2"], ct["LT_b2k"]),
            ]

            h1s_tiles = [None] * nt
            # two independent accumulation groups in one bank: tiles 0..7 at
            # base partition 0, tiles 8..15 at base 32 (matmul output base
            # partitions must be 0/32/64), so the first sqrt can run while
            # the second half of phase A is still streaming
            nsq_ps = psn_pool.tile([40, BT], F32, name="nsq_ps", tag="nsq")

            # ================= phase A: wavefront schedule =================
            # The per-tile chain is split into 11 macro-stages; tile t runs
            # stage m at wave t+m, so at steady state every wave carries one
            # tile in each stage.  Ops are emitted per wave in per-engine
            # readiness order, which makes each in-order engine queue process
            # ops exactly as their inputs become available: PE ~16 matmul
            # slots, ACT 5 ops, DVE 4 ops, GpSimd 2 ops per wave -- all four
            # engines near their ~3.4us/tile budget with no cross-pair
            # serialization.
            st = {t: {} for t in range(nt)}  # per-tile in-flight tensors

            def mm_T(t, lt_top, lt_bot, key_in, with_z):
                s = st[t]
                pv = pv_pool.tile([H, BT], F32, name="pv", tag="pv")
                ycat = s[key_in]
                nc.tensor.matmul(pv[:], lt_top[:], ycat[:, 0, :], start=True, stop=False)
                nc.tensor.matmul(pv[:], lt_bot[:], ycat[:, 1, :], start=False,
                                 stop=not with_z)
                if with_z:
                    nc.tensor.matmul(pv[:], ct["LT_z"][:], s["z"][0:D, :],
                                     start=False, stop=True)
                return pv

            def w_T2mm(t):
                st[t]["pv2"] = mm_T(t, ct["LT_t2"], ct["LT_b2k"], "y3", True)

            def w_ybc(t, key_pv, key_out):
                s = st[t]
                ycat = ycat_pool.tile([H, 2, BT], BF16, name="ycat")
                nc.vector.tensor_tensor(
                    ycat[:], s["acat"][:],
                    s.pop(key_pv)[:, None, :].broadcast_to([H, 2, BT]),
                    mybir.AluOpType.mult,
                )
                s[key_out] = ycat

            def w_T4mm(t):
                st[t]["pv4"] = mm_T(t, ct["LT_t4"], ct["LT_b4"], "y0", True)

            def w_acat(t):
                s = st[t]
                acat = acat_pool.tile([H, 2, BT], BF16, name="acat")
                nc.scalar.activation(acat[:, 0, :], s.pop("apt")[:], AF.Identity,
                                     bias=ct["Bse2t"][:])
                nc.scalar.activation(acat[:, 1, :], s.pop("apb")[:], AF.Identity,
                                     bias=ct["Bse2b"][:])
                s["acat"] = acat

            def w_pvb(t):
                s = st[t]
                pvb = pvb_pool.tile([H, BT], BF16, name="pvb")
                nc.scalar.activation(pvb[:], s.pop("pv4")[:], AF.Identity)
                s["pvb"] = pvb

            def w_y2d(t):
                # all-bf16 SBUF multiply: one op, broadcast pvb over halves
                s = st[t]
                pvb = s.pop("pvb")
                ycat = ycat_pool.tile([H, 2, BT], BF16, name="ycat")
                nc.vector.tensor_tensor(
                    ycat[:], s["acat"][:],
                    pvb[:, None, :].broadcast_to([H, 2, BT]),
                    mybir.AluOpType.mult,
                )
                s["y4"] = ycat

            def w_T3mm(t):
                st[t]["pv3"] = mm_T(t, ct["LT_t3"], ct["LT_b3"], "y4", True)

            def w_H1mm(t):
                s = st[t]
                h1p = pv_pool.tile([H, BT], F32, name="h1p", tag="pv")
                ycat = s.pop("y2")
                nc.tensor.matmul(h1p[:], ct["LT_t1"][:], ycat[:, 0, :], start=True, stop=False)
                nc.tensor.matmul(h1p[:], ct["LT_b1k"][:], ycat[:, 1, :], start=False, stop=False)
                nc.tensor.matmul(h1p[:], ct["LT_W1z"][:], s["z"][0:D, :],
                                 start=False, stop=True)
                s["h1p"] = h1p
                s.pop("acat")

            def w_h1s(t):
                s = st[t]
                h1s = h1s_pool.tile([H, BT], BF16, name="h1s")
                nc.scalar.activation(h1s[:], s.pop("h1p")[:], AF.Identity, bias=ct["B1"][:])
                h1s_tiles[t] = h1s

            def w_sq(t):
                # square on GpSimd from the bf16 h1 (both operands SBUF)
                sq = sq_pool.tile([H, BT], BF16, name="sq")
                nc.gpsimd.tensor_tensor(sq[:], h1s_tiles[t][:], h1s_tiles[t][:],
                                        mybir.AluOpType.mult)
                st[t]["sq"] = sq

            def w_nsq(t):
                out = nsq_ps[0:hnt, :] if t < hnt else nsq_ps[32:32 + hnt, :]
                nc.tensor.matmul(
                    out, onsq_t[:, bass.ts(t, hnt)], st[t].pop("sq")[:],
                    start=(t % hnt == 0), stop=(t % hnt == hnt - 1),
                    skip_group_check=True,
                )

            def w_apmm(t):
                s = st[t]
                apt = ps_pool.tile([H, BT], F32, name="apt", tag="ps")
                nc.tensor.matmul(apt[:], ct["LT_At"][:], s["hs"][:], start=True, stop=True)
                apb = ps_pool.tile([H, BT], F32, name="apb", tag="ps")
                nc.tensor.matmul(apb[:], ct["LT_Ab"][:], s.pop("hs")[:], start=True, stop=True)
                s["apt"], s["apb"] = apt, apb

            def w_hpmm(t):
                s = st[t]
                hp = ps_pool.tile([H, BT], F32, name="hp", tag="ps")
                nc.tensor.matmul(hp[:], ct["LT_h"][:], s["z"][0:D, :], start=True, stop=True)
                s["hp"] = hp

            def w_silu(t):
                s = st[t]
                hp = s.pop("hp")
                hs = hs_pool.tile([H, BT], BF16, name="hs")
                if sim_safe:
                    sg = hs_pool.tile([H, BT], F32, name="sg")
                    nc.scalar.activation(sg[:], hp[:], AF.Sigmoid, bias=ct["Bse1"][:])
                    hx = hs_pool.tile([H, BT], F32, name="hx")
                    nc.scalar.activation(hx[:], hp[:], AF.Identity, bias=ct["Bse1"][:])
                    nc.vector.tensor_tensor(hs[:], sg[:], hx[:], mybir.AluOpType.mult)
                else:
                    nc.scalar.activation(hs[:], hp[:], AF.Silu, bias=ct["Bse1"][:])
                s["hs"] = hs

            def w_ycat0(t):
                # all-SBUF bf16 multiply: runs on GpSimd to keep DVE free
                # for the PSUM-sourced taylor multiplies
                s = st[t]
                ycat = ycat_pool.tile([H, 2, BT], BF16, name="ycat0")
                nc.gpsimd.tensor_tensor(
                    ycat[:], s["acat"][:],
                    s["z"][:, None, :].broadcast_to([H, 2, BT]),
                    mybir.AluOpType.mult,
                )
                s["y0"] = ycat

            def w_dma(t):
                # pair-shared input load (4 replica DMAs per 2 tiles)
                zv4 = zv4_pool.tile([H, 2 * BT], BF16, name="zv4")
                for r in range(4):
                    nc.sync.dma_start(
                        zv4[32 * r:32 * (r + 1), :],
                        zT[:, bass.ts(t // 2, 2 * BT)],
                    )
                st[t]["z"] = zv4[:, 0:BT]
                st[t + 1]["z"] = zv4[:, BT:2 * BT]

            # wave w, tile w-m runs macro-stage m:
            #  m0 dma | m1 hp,silu | m2 ap | m3 acat | m4 ycat0
            #  m5 T4,pvb,y2d | m6 T3 | m7 ybc(pv3) | m8 T2,ybc(pv2)
            #  m9 H1,h1s | m10 sq | m11 nsq
            def alive(m):
                t = w - m
                return t if 0 <= t < nt else None

            rt_a = gate_pool.tile([hnt, BT], F32, name="rt_a")
            rt_b = gate_pool.tile([hnt, BT], F32, name="rt_b")

            for w in range(nt + 12):
                if 1 <= w <= 11:
                    warm(3)
                if w == nt + 5:
                    # first-half norms closed (wave nt+3); the switch to the
                    # sqrt table and this sqrt hide under the phase A tail --
                    # the only remaining phase A ACT ops are Identity, which
                    # is filler in every table set
                    nc.scalar.activation(rt_a[:], nsq_ps[0:hnt, :], AF.Sqrt,
                                         bias=zero_b[:])
                if (t := alive(0)) is not None and t % 2 == 0:
                    w_dma(t)
                if (t := alive(8)) is not None:
                    w_T2mm(t)
                if (t := alive(7)) is not None:
                    w_ybc(t, "pv3", "y3")
                if (t := alive(8)) is not None:
                    w_ybc(t, "pv2", "y2")
                if (t := alive(5)) is not None:
                    w_T4mm(t)
                if (t := alive(3)) is not None:
                    w_acat(t)
                if (t := alive(5)) is not None:
                    w_pvb(t)
                    w_y2d(t)
                if (t := alive(6)) is not None:
                    w_T3mm(t)
                if (t := alive(9)) is not None:
                    w_H1mm(t)
                    w_h1s(t)
                if (t := alive(4)) is not None:
                    w_ycat0(t)
                if (t := alive(10)) is not None:
                    w_sq(t)
                if (t := alive(11)) is not None:
                    w_nsq(t)
                if (t := alive(2)) is not None:
                    w_apmm(t)
                if (t := alive(1)) is not None:
                    w_hpmm(t)
                    w_silu(t)

            # ============== gate tail (second sqrt + both tanhs) ==============
            # sqrt set is already loaded (first sqrt ran under the phase A
            # tail); one switch back to a tanh set covers both tanh calls
            # and all of phase B (Identity is filler everywhere).  Bridge
            # the remaining ACT-serial bubble (~2.5us) with warm matmuls --
            # no more, or they delay the gate matmuls in the PE queue.
            warm(8)
            nc.scalar.activation(rt_b[:], nsq_ps[32:32 + hnt, :], AF.Sqrt,
                                 bias=zero_b[:])
            # sigmoid(norm + eps) = 0.5 tanh(0.5 norm + eps/2) + 0.5
            nc.scalar.activation(t9a[0:hnt, :], rt_a[:], AF.Tanh, bias=tanh_b[:], scale=0.5)
            nc.scalar.activation(t9b[0:hnt, :], rt_b[:], AF.Tanh, bias=tanh_b[:], scale=0.5)

            # ================= phase B (software-pipelined) =================
            trps = {}

            def emit_trp(t):
                trp = ps_pool.tile([H, BT], F32, name="trp", tag="ps")
                nc.tensor.matmul(
                    trp[:], esig_t[:, bass.ts(t, H)],
                    t9a[:] if t < hnt else t9b[:],
                    start=True, stop=True,
                )
                trps[t] = trp

            for t in range(min(4, nt)):
                emit_trp(t)
            outg = None
            for t in range(nt):
                g, r = divmod(t, 2)
                a1g = a1g_pool.tile([H, BT], BF16, name="a1g")
                nc.vector.tensor_tensor(
                    a1g[:], h1s_tiles[t][:], trps.pop(t)[:], mybir.AluOpType.mult
                )
                if r == 0:
                    outg = pv_pool.tile([H, BT], F32, name="outg", tag="pv")
                nc.tensor.matmul(
                    outg[32 * r:32 * (r + 1), :], ct["LT_W2"][:], a1g[:],
                    start=True, stop=True, skip_group_check=True,
                )
                if t + 4 < nt:
                    emit_trp(t + 4)
                if r == 1:
                    outs = outs_pool.tile([2 * D, BT], F32, name="outs")
                    nc.scalar.activation(outs[:], outg[0:2 * D, :], AF.Identity, bias=b22_t[:])
                    # alternate queues so the final stores drain in parallel
                    eng = nc.gpsimd if g % 2 == 0 else nc.sync
                    eng.dma_start(outT[:, bass.ts(g, BT)], outs[:])

    if split_waits:
        _split_multi_waits(nc)
    return nc


def _host_params(G, W_se1, b_se1, W_se2, b_se2, W1, b1, W2, b2, nt):
    import ml_dtypes
    f = np.float32
    bf = ml_dtypes.bfloat16
    G = np.asarray(G, f)
    Gflat = np.transpose(G, (0, 2, 1)).reshape(NG * D, D)  # [(g,i), j] = G[g,j,i]
    W1G = Gflat @ np.asarray(W1, f).T                      # [(g,i), m]
    hnt = nt // 2
    e_sig = np.zeros((hnt + 1, nt * H), f)
    for t in range(nt):
        e_sig[t % hnt, t * H:(t + 1) * H] = 0.5
    e_sig[hnt, :] = 0.5
    onsq2 = np.zeros((H, nt * hnt), f)
    for t in range(nt):
        onsq2[:, t * hnt + (t % hnt)] = 1.0
    w = {
        "LT_h": np.asarray(W_se1, f).T,
        "LT_At": np.repeat(np.asarray(W_se2, f).T[:, 0:4], 32, axis=1),
        "LT_Ab": np.repeat(np.asarray(W_se2, f).T[:, 4:8], 32, axis=1),
        "Bse1": np.asarray(b_se1, f).reshape(H, 1),
        "Bse2t": np.repeat(np.asarray(b_se2, f)[0:4], 32).reshape(H, 1),
        "Bse2b": np.repeat(np.asarray(b_se2, f)[4:8], 32).reshape(H, 1),
        "LT_z": np.tile(np.eye(D, dtype=f), (1, 4)),
        "LT_W1z": np.asarray(W1, f).T,
        "B1": np.asarray(b1, f).reshape(H, 1),
        "LT_W2": np.asarray(W2, f).T,
        "LT_t1": np.ascontiguousarray(W1G[:H]),
        "LT_b1k": np.ascontiguousarray(W1G[H:]),
    }
    for k, tname, bname in ((4, "LT_t4", "LT_b4"), (3, "LT_t3", "LT_b3"), (2, "LT_t2", "LT_b2k")):
        scaled = np.tile(Gflat * f(1.0 / k), (1, 4))
        w[tname] = np.ascontiguousarray(scaled[:H])
        w[bname] = np.ascontiguousarray(scaled[H:])
    p = {
        "CAT128": np.concatenate([w[n] for n, _ in _CAT128], axis=1).astype(bf),
        "CAT32": np.concatenate([w[n] for n, _ in _CAT32], axis=1).astype(bf),
        "CATB": np.concatenate([w[n] for n, _ in _CATB], axis=1).astype(f),
        "ONSQ": onsq2.astype(bf),
        "E_sig": e_sig.astype(bf),
        "B2_2": np.tile(np.asarray(b2, f), 2).reshape(2 * D, 1).astype(f),
    }
    return {k: np.ascontiguousarray(v) for k, v in p.items()}


def _run(z, G, W_se1, b_se1, W_se2, b_se2, W1, b1, W2, b2, trace=False, **trace_kw):
    import ml_dtypes
    z = np.asarray(z, np.float32)
    nt = BC // BT
    params = _host_params(G, W_se1, b_se1, W_se2, b_se2, W1, b1, W2, b2, nt)

    # shard: per-core feature-major bf16 slices
    zT = np.ascontiguousarray(
        z.reshape(NCORES, BC, D).transpose(0, 2, 1).astype(ml_dtypes.bfloat16)
    )

    nc = _build_program(BC)
    in_maps = [{"zT": zT[c], **params} for c in range(NCORES)]
    res = run_bass_kernel_spmd(nc, in_maps, list(range(NCORES)), trace=trace, **trace_kw)

    # outT[32r+d, g*BT+b] = out[(2g+r)*BT + b, d] per core
    outT = np.stack([res.results[c]["outT"] for c in range(NCORES)])
    out = (
        outT.reshape(NCORES, 2, D, nt // 2, BT)
        .transpose(0, 3, 1, 4, 2)
        .reshape(B, D)
    )
    return np.ascontiguousarray(out.astype(np.float32)), res


def kernel(z, G, W_se1, b_se1, W_se2, b_se2, W1, b1, W2, b2):
    out, _ = _run(z, G, W_se1, b_se1, W_se2, b_se2, W1, b1, W2, b2, trace=False)
    return out


if __name__ == "__main__":
    rng = np.random.default_rng(0)
    inputs = {
        "z": rng.standard_normal((B, D), dtype=np.float32),
        "G": (rng.standard_normal((NG, D, D)) * 0.1).astype(np.float32),
        "W_se1": (rng.standard_normal((H, D)) / np.sqrt(D)).astype(np.float32),
        "b_se1": np.zeros(H, np.float32),
        "W_se2": (rng.standard_normal((NG, H)) / np.sqrt(H)).astype(np.float32),
        "b_se2": np.zeros(NG, np.float32),
        "W1": (rng.standard_normal((H, D)) * 0.01).astype(np.float32),
        "b1": np.zeros(H, np.float32),
        "W2": (rng.standard_normal((D, H)) * 0.01).astype(np.float32),
        "b2": np.zeros(D, np.float32),
    }
    out = kernel(**inputs)
    print("kernel output", out.shape, out.dtype, float(np.abs(out).max()))
